# revision 2
# baseline (speedup 1.0000x reference)
"""Causal MHA (B=4, S=2048, D=1024, H=16, RoPE) on 8 trn2 cores — v2.

Sharding: core c -> batch c//2, head-half c%2 (8 heads / 512 dims per core).

Design vs v1 baseline:
  - Q/K weights host-permuted into even/odd 32-blocks per 4-head group so
    RoPE needs NO partition-swap DMA: psum tile pairs (j, j+1) hold the
    even/odd halves at identical partition indices and the rotation is plain
    elementwise tensor ops (fp16, 2x DVE rate).
  - fp16 downstream (P, V, attn, Wo): 1 cyc/row matmuls.
  - Optional fp8e4m3 DoubleRow scores (0.5 cyc/row): the even/odd layout is
    natively DR-compatible ([32 parts, 2 j-slots, t] APs), no re-layout DMA.
  - PV in [q, dv] orientation (moving dim 65): psum accumulators
    [128, 2qt, 130] with ones-column densities; normalization via
    per-partition recips + diag-matmul transpose back to [d', t].
  - Software-pipelined emission: normalize(m) deferred into (m+1)'s kt loop,
    O-proj(qb) into qb+1, phase-A chunks interleaved at m boundaries, so the
    in-order PE queue never head-of-line blocks on DVE/Act chains.
"""

import numpy as np

import concourse.bass as bass
import concourse.bacc as bacc
import concourse.mybir as mybir
import concourse.tile as tile
from concourse.bass import ds, ts
from concourse.bass_utils import run_bass_kernel_spmd

F32 = mybir.dt.float32
F16 = mybir.dt.float16
F8 = mybir.dt.float8e4
DR = mybir.MatmulPerfMode.DoubleRow

B, S, D, H, DK = 4, 2048, 1024, 16, 64
THETA = 10000.0
NH = 8
HD = NH * DK
P = 128
NEG = -28000.0
NEG8 = -240.0

CFG = {
    "s_dr": "ge512",   # "none" | "ge512" | "all"
    "proj_dr": False,
    "rope_mul_pool": False,
}


def build_attention_nc(nrep=1):
    nc = bacc.Bacc("TRN2", target_bir_lowering=False, debug=False)

    xh = nc.dram_tensor("xh", [D // P, P, S], F16, kind="ExternalInput")
    wqh = nc.dram_tensor("wqh", [D // P, P, HD], F16, kind="ExternalInput")
    wkh = nc.dram_tensor("wkh", [D // P, P, HD], F16, kind="ExternalInput")
    wvh = nc.dram_tensor("wvh", [D // P, P, HD], F16, kind="ExternalInput")
    woh = nc.dram_tensor("woh", [HD // P, P, D], F16, kind="ExternalInput")
    cosP = nc.dram_tensor("cosP", [P, S], F16, kind="ExternalInput")
    sinP = nc.dram_tensor("sinP", [P, S], F16, kind="ExternalInput")
    ident = nc.dram_tensor("ident", [P, P], F16, kind="ExternalInput")
    umask = nc.dram_tensor("umask", [P, 512], F16, kind="ExternalInput")
    negI = nc.dram_tensor("negI", [P, P], F16, kind="ExternalInput")
    umask8 = nc.dram_tensor("umask8", [P, 2, 512], F8, kind="ExternalInput")
    negI8 = nc.dram_tensor("negI8", [P, 2, P], F8, kind="ExternalInput")
    outT = nc.dram_tensor("outT", [D, S], F32, kind="ExternalOutput")
    if CFG.get("debug"):
        dQT = nc.dram_tensor("dQT", [P, 4, S], F16, kind="ExternalOutput")
        dKT = nc.dram_tensor("dKT", [P, 4, S], F16, kind="ExternalOutput")
        dVP = nc.dram_tensor("dVP", [P, 16, NH, DK + 1], F16, kind="ExternalOutput")
        dHOPT = nc.dram_tensor("dHOPT", [P, 4, S], F16, kind="ExternalOutput")
        CFG["_dbg"] = (dQT, dKT, dVP, dHOPT)

    if CFG["proj_dr"]:
        x8 = nc.dram_tensor("x8", [D // 256, P, 2, S], F8, kind="ExternalInput")
        wq8 = nc.dram_tensor("wq8", [D // 256, P, 2, HD], F8, kind="ExternalInput")
        wk8 = nc.dram_tensor("wk8", [D // 256, P, 2, HD], F8, kind="ExternalInput")
    else:
        x8 = wq8 = wk8 = None

    with tile.TileContext(nc) as tc:
        if nrep == 1:
            _attn_tile(tc, xh, x8, wqh, wkh, wq8, wk8, wvh, woh, cosP, sinP,
                       ident, umask, negI, umask8, negI8, outT)
        else:
            with tc.For_i(0, nrep, 1):
                _attn_tile(tc, xh, x8, wqh, wkh, wq8, wk8, wvh, woh, cosP,
                           sinP, ident, umask, negI, umask8, negI8, outT)
    nc.compile()
    return nc


def _attn_tile(tc, xh, x8, wqh, wkh, wq8, wk8, wvh, woh, cosP, sinP, ident,
               umask, negI, umask8, negI8, outT):
    nc = tc.nc
    s_dr = CFG["s_dr"]
    proj_dr = CFG["proj_dr"]
    need_f8 = s_dr != "none"
    need_f16qt = s_dr != "all"

    with (
        tc.tile_pool(name="per", bufs=1) as per,
        tc.tile_pool(name="ab", bufs=6) as abp,
        tc.tile_pool(name="rt", bufs=4) as rtp,
        tc.tile_pool(name="ptp", bufs=3) as ptp,
        tc.tile_pool(name="arp", bufs=4) as arp,
        tc.tile_pool(name="rcp", bufs=2) as rcp,
        tc.tile_pool(name="dgp", bufs=4) as dgp,
        tc.tile_pool(name="obp", bufs=2) as obp,
        tc.tile_pool(name="psS", bufs=2, space="PSUM") as psS,
        tc.tile_pool(name="psPV", bufs=2, space="PSUM") as psPV,
        tc.tile_pool(name="psC", bufs=2, space="PSUM") as psC,
    ):
        # ---------------- persistent tiles + input DMAs ----------------
        if need_f16qt:
            QT = per.tile([P, 4, S], F16, tag="QT")
            KT = per.tile([P, 4, S], F16, tag="KT")
        else:
            QT = KT = None
        if need_f8:
            QT8 = per.tile([P, 4, S], F8, tag="QT8")
            KT8 = per.tile([P, 4, S], F8, tag="KT8")
        else:
            QT8 = KT8 = None
        VP = per.tile([P, 16, NH, DK + 1], F16, tag="VP")
        HOPT = per.tile([P, 4, S], F16, tag="HOPT")

        # ones column of VP (col 0 per head)
        nc.vector.memset(VP[:, :, :, 0:1], 1.0)

        # DMAs ordered so phase_a(0, 0) can start as early as possible:
        # j-pair-0 halves of wq/wk first, then the first x chunk, cos/sin, wv.
        xh_sb = per.tile([P, 8, S], F16, tag="xh")
        xh_src = xh.ap().rearrange("o p t -> p o t")
        if proj_dr:
            x8_sb = per.tile([P, 4, 2, S], F8, tag="x8")
            wq_sb = per.tile([P, 4, 2, HD], F8, tag="wq")
            wk_sb = per.tile([P, 4, 2, HD], F8, tag="wk")
            wq_src = wq8.ap().rearrange("c p i d -> p c i d")
            wk_src = wk8.ap().rearrange("c p i d -> p c i d")
            for jp in (0, 1):
                dsl = ds(jp * 256, 256)
                nc.sync.dma_start(wq_sb[:, :, :, dsl], wq_src[:, :, :, dsl])
                nc.sync.dma_start(wk_sb[:, :, :, dsl], wk_src[:, :, :, dsl])
        else:
            wq_sb = per.tile([P, 8, HD], F16, tag="wq")
            wk_sb = per.tile([P, 8, HD], F16, tag="wk")
            wq_src = wqh.ap().rearrange("o p d -> p o d")
            wk_src = wkh.ap().rearrange("o p d -> p o d")
            for jp in (0, 1):
                dsl = ds(jp * 256, 256)
                nc.sync.dma_start(wq_sb[:, :, dsl], wq_src[:, :, dsl])
                nc.sync.dma_start(wk_sb[:, :, dsl], wk_src[:, :, dsl])
        nc.sync.dma_start(xh_sb[:, :, 0:512], xh_src[:, :, 0:512])
        cos_sb = per.tile([P, S], F16, tag="cos")
        sin_sb = per.tile([P, S], F16, tag="sin")
        nc.sync.dma_start(cos_sb, cosP.ap())
        nc.sync.dma_start(sin_sb, sinP.ap())
        wv_sb = per.tile([P, 8, HD], F16, tag="wv")
        nc.sync.dma_start(wv_sb, wvh.ap().rearrange("o p d -> p o d"))
        for tci in range(1, 4):
            tsl = ds(tci * 512, 512)
            nc.sync.dma_start(xh_sb[:, :, tsl], xh_src[:, :, tsl])
        if proj_dr:
            nc.sync.dma_start(x8_sb, x8.ap().rearrange("c p i t -> p c i t"))
        wo_sb = per.tile([P, 4, D], F16, tag="wo")
        nc.sync.dma_start(wo_sb, woh.ap().rearrange("m p o -> p m o"))
        I_sb = per.tile([P, P], F16, tag="I")
        nc.sync.dma_start(I_sb, ident.ap())
        z_sb = per.tile([P, 512], F16, tag="z")
        nc.vector.memset(z_sb[0:1, :], 0.0)
        um_sb = per.tile([P, 512], F16, tag="um")
        nc.sync.dma_start(um_sb, umask.ap())
        nI_sb = per.tile([P, P], F16, tag="nI")
        nc.sync.dma_start(nI_sb, negI.ap())
        if need_f8:
            um8_sb = per.tile([P, 2, 512], F8, tag="um8")
            nc.sync.dma_start(um8_sb, umask8.ap())
            nI8_sb = per.tile([P, 2, P], F8, tag="nI8")
            nc.sync.dma_start(nI8_sb, negI8.ap())

        outT_ap = outT.ap()

        # ---------------- phase A half-chunk (t block 512, one j-pair) ----
        def phase_a(tci, jp):
            tsl = ds(tci * 512, 512)
            abt = []
            for w in (wq_sb, wk_sb):
                for j in (2 * jp, 2 * jp + 1):
                    ps = psS.tile([P, 512], F32, tag="s", name=f"pa{tci}{j}")
                    if proj_dr:
                        for c in range(4):
                            nc.tensor.matmul(
                                ps,
                                lhsT=w[:, c, :, ts(j, P)],
                                rhs=x8_sb[:, c, :, tsl],
                                start=(c == 0), stop=(c == 3),
                                perf_mode=DR,
                            )
                    else:
                        for o in range(8):
                            nc.tensor.matmul(
                                ps,
                                lhsT=w[:, o, ts(j, P)],
                                rhs=xh_sb[:, o, tsl],
                                start=(o == 0), stop=(o == 7),
                            )
                    ab = abp.tile([P, 512], F16, tag="ab", name=f"ab{j}")
                    nc.vector.tensor_copy(ab, ps)
                    abt.append(ab)
            # V projection for two t-subtiles
            for tt in (2 * jp, 2 * jp + 1):
                psv = psS.tile([P, 512], F32, tag="s", name=f"pvv{tci}{tt}")
                for o in range(8):
                    nc.tensor.matmul(
                        psv,
                        lhsT=xh_sb[:, o, ds(tci * 512 + tt * P, P)],
                        rhs=wv_sb[:, o, :],
                        start=(o == 0), stop=(o == 7),
                    )
                nc.vector.tensor_copy(
                    VP[:, tci * 4 + tt, :, 1:],
                    psv.rearrange("p (h c) -> p h c", c=DK),
                )
            # RoPE for this j-pair, both tensors
            mul_eng = nc.gpsimd if CFG["rope_mul_pool"] else nc.vector
            for tensor in (0, 1):
                A, Bb = abt[2 * tensor], abt[2 * tensor + 1]
                dst = (QT, KT)[tensor]
                dst8 = (QT8, KT8)[tensor]
                je, jo = 2 * jp, 2 * jp + 1
                t1 = rtp.tile([P, 512], F16, tag="rt", name="t1")
                t2 = rtp.tile([P, 512], F16, tag="rt", name="t2")
                mul_eng.tensor_mul(t1, cos_sb[:, tsl], A)
                mul_eng.tensor_mul(t2, sin_sb[:, tsl], Bb)
                t3 = rtp.tile([P, 512], F16, tag="rt", name="t3")
                t4 = rtp.tile([P, 512], F16, tag="rt", name="t4")
                mul_eng.tensor_mul(t3, sin_sb[:, tsl], A)
                mul_eng.tensor_mul(t4, cos_sb[:, tsl], Bb)
                if need_f16qt:
                    nc.vector.tensor_sub(dst[:, je, tsl], t1, t2)
                    nc.vector.tensor_add(dst[:, jo, tsl], t3, t4)
                    if need_f8:
                        nc.gpsimd.tensor_copy(dst8[:, je, tsl], dst[:, je, tsl])
                        nc.gpsimd.tensor_copy(dst8[:, jo, tsl], dst[:, jo, tsl])
                else:
                    nc.vector.tensor_sub(dst8[:, je, tsl], t1, t2)
                    nc.vector.tensor_add(dst8[:, jo, tsl], t3, t4)

        # ---------------- attention stages ----------------
        def kt_loop(qb, m):
            """S + exp + PV accumulation for head pair m of q block qb.
            Returns state consumed by normalize()."""
            use_dr = s_dr == "all" or (s_dr == "ge512" and qb >= 1)
            pvt = [
                psPV.tile([P, 2, 130], F32, tag="pv", name=f"pv{qb}{m}{i}")
                for i in range(2)
            ]
            # start=True zeroes the whole 2KB bank (ZERO_REGION_SIZE), so a
            # shared-bank tile is zeroed once up front and all sub-region
            # accumulations use start=False.
            for i in range(2):
                nc.tensor.matmul(
                    pvt[i], lhsT=z_sb[0:1, 0:P], rhs=z_sb[0:1, 0:260],
                    start=True, stop=False, skip_group_check=True,
                )
            nkt = 4 * qb + 4
            for kt in range(nkt):
                roff = kt - 4 * qb
                c0 = P * max(roff, 0)
                live = 512 - c0
                s2 = psS.tile([P, 2, 512], F32, tag="s", name=f"s{qb}{m}{kt}")
                for e in (0, 1):
                    h = 2 * m + e
                    g, h4 = h // 4, h % 4
                    pr = ds(h4 * 32, 32)
                    tpos = (h4 * 32, 0)
                    if use_dr:
                        nc.tensor.matmul(
                            s2[:, e, c0:],
                            lhsT=KT8[pr, ds(2 * g, 2), ts(kt, P)],
                            rhs=QT8[pr, ds(2 * g, 2), ds(qb * 512 + c0, live)],
                            start=True, stop=(roff < 0),
                            perf_mode=DR,
                            skip_group_check=True,
                            tile_position=tpos,
                        )
                    else:
                        nc.tensor.matmul(
                            s2[:, e, c0:],
                            lhsT=KT[pr, 2 * g, ts(kt, P)],
                            rhs=QT[pr, 2 * g, ds(qb * 512 + c0, live)],
                            start=True, stop=False,
                            skip_group_check=True,
                            tile_position=tpos,
                        )
                        nc.tensor.matmul(
                            s2[:, e, c0:],
                            lhsT=KT[pr, 2 * g + 1, ts(kt, P)],
                            rhs=QT[pr, 2 * g + 1, ds(qb * 512 + c0, live)],
                            start=False, stop=(roff < 0),
                            skip_group_check=True,
                            tile_position=tpos,
                        )
                    if roff >= 0:
                        if use_dr:
                            nc.tensor.matmul(
                                s2[:, e, ds(c0, P)],
                                lhsT=nI8_sb,
                                rhs=um8_sb[:, :, 0:P],
                                start=False, stop=True,
                                perf_mode=DR,
                                skip_group_check=True,
                            )
                        else:
                            nc.tensor.matmul(
                                s2[:, e, ds(c0, P)],
                                lhsT=nI_sb,
                                rhs=um_sb[:, 0:P],
                                start=False, stop=True,
                                skip_group_check=True,
                            )
                pt2 = ptp.tile([P, 2, 512], F16, tag="pt")
                nc.scalar.activation(
                    pt2[:, :, c0:], s2[:, :, c0:],
                    mybir.ActivationFunctionType.Exp, scale=0.125,
                )
                for e in (0, 1):
                    h = 2 * m + e
                    for qt in range(4):
                        if qt < roff:
                            continue
                        nc.tensor.matmul(
                            pvt[qt // 2][:, qt % 2, ds(e * 65, 65)],
                            lhsT=pt2[:, e, ds(qt * P, P)],
                            rhs=VP[:, kt, h, :],
                            start=False,
                            stop=(kt == 4 * qb + qt),
                            skip_group_check=True,
                        )
            return pvt

        def norm_a(qb, m, pvt):
            """Drain pv psum right away: recip denominators to SBUF + raw
            fp16 copy. DVE-only, so the PE queue is not blocked; frees the
            pv psum ring for the next kt_loop."""
            rc = rcp.tile([P, 2, 2, 2], F32, tag="rc", name=f"rc{qb}{m}")
            araw = []
            for i in (0, 1):
                rsl = pvt[i].rearrange("p a (b c) -> p a b c", c=65)[:, :, :, 0]
                nc.vector.reciprocal(rc[:, i], rsl)
                ar = arp.tile([P, 2, 130], F16, tag="ar", name=f"ar{i}")
                nc.vector.tensor_copy(ar, pvt[i])
                araw.append(ar)
            return rc, araw

        def norm_b(qb, m, rc, araw):
            """Deferred: diag builds + scaled transpose into HOPT."""
            qsl = ds(qb * 512, 512)
            tp = psC.tile([P, 512], F32, tag="c", name=f"tp{qb}{m}")
            nc.tensor.matmul(
                tp, lhsT=z_sb[0:1, 0:P], rhs=z_sb[0:1, :],
                start=True, stop=False, skip_group_check=True,
            )
            for qt in range(4):
                for e in (0, 1):
                    dg = dgp.tile([P, P], F16, tag="dg")
                    nc.vector.tensor_scalar_mul(
                        dg, I_sb, rc[:, qt // 2, qt % 2, ds(e, 1)]
                    )
                    nc.tensor.matmul(
                        tp[ds(e * DK, DK), ts(qt, P)],
                        lhsT=araw[qt // 2][:, qt % 2, ds(e * 65 + 1, DK)],
                        rhs=dg,
                        start=False, stop=(qt == 3),
                        skip_group_check=True,
                    )
            nc.vector.tensor_copy(HOPT[:, m, qsl], tp)

        def o_proj(qb):
            qsl = ds(qb * 512, 512)
            for og in range(2):
                ob = obp.tile([P, 4, 512], F32, tag="ob")
                for oi in range(4):
                    ot = og * 4 + oi
                    psc = psC.tile([P, 512], F32, tag="c", name=f"oc{qb}{ot}")
                    for mm in range(4):
                        nc.tensor.matmul(
                            psc,
                            lhsT=wo_sb[:, mm, ts(ot, P)],
                            rhs=HOPT[:, mm, qsl],
                            start=(mm == 0), stop=(mm == 3),
                        )
                    nc.vector.tensor_copy(ob[:, oi, :], psc)
                nc.sync.dma_start(
                    outT_ap[ds(og * 512, 512), qsl].rearrange(
                        "(o p) q -> p o q", p=P
                    ),
                    ob,
                )

        # ---------------- emission schedule (software-pipelined) ----------
        # pending normalize/o_proj work is emitted one stage later so the
        # in-order PE queue always has ready S/PV work at its head.
        phase_a(0, 0)
        phase_a(0, 1)

        pend_norm = None   # (qb, m, rc, araw)
        pend_oproj = None  # qb

        # extra phase-A half-chunks interleaved at (qb, m) boundaries
        a_sched = {
            (0, 0): (1, 0), (0, 1): (1, 1),
            (0, 2): (2, 0), (0, 3): (2, 1),
            (1, 0): (3, 0), (1, 1): (3, 1),
        }

        for qb in range(4):
            for m in range(4):
                pvt = kt_loop(qb, m)
                rc, araw = norm_a(qb, m, pvt)
                if pend_norm is not None:
                    norm_b(*pend_norm)
                pend_norm = (qb, m, rc, araw)
                if (qb, m) in a_sched:
                    phase_a(*a_sched[(qb, m)])
                if pend_oproj is not None and m == 1:
                    o_proj(pend_oproj)
                    pend_oproj = None
            pend_oproj = qb
        norm_b(*pend_norm)
        o_proj(3)

        if CFG.get("debug"):
            dQT, dKT, dVP, dHOPT = CFG["_dbg"]
            nc.sync.dma_start(dQT.ap(), QT if need_f16qt else QT8)
            nc.sync.dma_start(dKT.ap(), KT if need_f16qt else KT8)
            nc.sync.dma_start(dHOPT.ap(), HOPT)
            nc.sync.dma_start(dVP.ap(), VP)


# ---------------- host side ----------------

def _qk_perm():
    perm = []
    for j in range(4):
        for h4 in range(4):
            h = (j // 2) * 4 + h4
            par = j % 2
            perm.extend(h * 64 + 2 * np.arange(32) + par)
    return np.array(perm)


_PERM = _qk_perm()


def _host_tables():
    import ml_dtypes

    F16n = np.float16
    i = np.arange(32, dtype=np.float32)
    inv_freq = (THETA ** (2.0 * i / DK)).astype(np.float32)
    t = np.arange(S, dtype=np.float32)
    ang = t[None, :] / inv_freq[:, None]          # [32, S]
    cosP = np.tile(np.cos(ang), (4, 1)).astype(F16n)   # [128, S]
    sinP = np.tile(np.sin(ang), (4, 1)).astype(F16n)
    ident = np.eye(P, dtype=F16n)
    kk = np.arange(P)[:, None]
    qq = np.arange(512)[None, :]
    umask = (kk > qq).astype(F16n)                # [128, 512]
    negI = (NEG * np.eye(P)).astype(F16n)
    F8n = ml_dtypes.float8_e4m3fn
    negI8 = np.zeros((P, 2, P), dtype=np.float32)
    umask8 = np.zeros((P, 2, 512), dtype=np.float32)
    for ii in range(2):
        for p in range(64):
            negI8[p, ii, ii * 64 + p] = NEG8
            umask8[p, ii, :] = ((ii * 64 + p) > qq[0]).astype(np.float32)
    return cosP, sinP, ident, umask, negI, umask8.astype(F8n), negI8.astype(F8n)


def make_in_maps(x, Wq, Wk, Wv, Wo):
    import ml_dtypes

    F16n = np.float16
    F8n = ml_dtypes.float8_e4m3fn
    cosP, sinP, ident, umask, negI, umask8, negI8 = _host_tables()
    in_maps = []
    for c in range(8):
        b, hh = c // 2, c % 2
        sl = slice(hh * HD, (hh + 1) * HD)
        xT = np.ascontiguousarray(x[b].T)                      # [1024, 2048]
        wq = Wq[sl, :][_PERM]
        wk = Wk[sl, :][_PERM]
        wv = Wv[sl, :]
        wo = Wo[:, sl]                                         # [1024, 512]
        woh = np.ascontiguousarray(wo.T).reshape(4, P, D)
        m = {
            "xh": xT.reshape(8, P, S).astype(F16n),
            "wqh": np.ascontiguousarray(wq.T).reshape(8, P, HD).astype(F16n),
            "wkh": np.ascontiguousarray(wk.T).reshape(8, P, HD).astype(F16n),
            "wvh": np.ascontiguousarray(wv.T).reshape(8, P, HD).astype(F16n),
            "woh": woh.astype(F16n),
            "cosP": cosP, "sinP": sinP, "ident": ident,
            "umask": umask, "negI": negI, "umask8": umask8, "negI8": negI8,
        }
        if CFG["proj_dr"]:
            m["x8"] = xT.reshape(4, 2, P, S).transpose(0, 2, 1, 3).astype(F8n)
            m["wq8"] = np.ascontiguousarray(wq.T).reshape(4, 2, P, HD).transpose(0, 2, 1, 3).astype(F8n)
            m["wk8"] = np.ascontiguousarray(wk.T).reshape(4, 2, P, HD).transpose(0, 2, 1, 3).astype(F8n)
        in_maps.append(m)
    return in_maps


def gather_out(core_outs):
    out = np.empty((B, S, D), dtype=np.float32)
    for b in range(B):
        out[b] = (core_outs[2 * b]["outT"] + core_outs[2 * b + 1]["outT"]).T
    return out


_NC_CACHE = {}


def kernel(x, Wq, Wk, Wv, Wo):
    x = np.asarray(x, dtype=np.float32)
    Wq = np.asarray(Wq, dtype=np.float32)
    Wk = np.asarray(Wk, dtype=np.float32)
    Wv = np.asarray(Wv, dtype=np.float32)
    Wo = np.asarray(Wo, dtype=np.float32)
    if "nc" not in _NC_CACHE:
        _NC_CACHE["nc"] = build_attention_nc()
    nc = _NC_CACHE["nc"]
    in_maps = make_in_maps(x, Wq, Wk, Wv, Wo)
    res = run_bass_kernel_spmd(nc, in_maps, core_ids=list(range(8)))
    return gather_out(res.results)


# revision 3
# speedup vs baseline: 1.1314x; 1.1314x over previous
"""Causal MHA (B=4, S=2048, D=1024, H=16, RoPE) on 8 trn2 cores — v2.

Sharding: core c -> batch c//2, head-half c%2 (8 heads / 512 dims per core).

Design vs v1 baseline:
  - Q/K weights host-permuted into even/odd 32-blocks per 4-head group so
    RoPE needs NO partition-swap DMA: psum tile pairs (j, j+1) hold the
    even/odd halves at identical partition indices and the rotation is plain
    elementwise tensor ops (fp16, 2x DVE rate).
  - fp16 downstream (P, V, attn, Wo): 1 cyc/row matmuls.
  - Optional fp8e4m3 DoubleRow scores (0.5 cyc/row): the even/odd layout is
    natively DR-compatible ([32 parts, 2 j-slots, t] APs), no re-layout DMA.
  - PV in [q, dv] orientation (moving dim 65): psum accumulators
    [128, 2qt, 130] with ones-column densities; normalization via
    per-partition recips + diag-matmul transpose back to [d', t].
  - Software-pipelined emission: normalize(m) deferred into (m+1)'s kt loop,
    O-proj(qb) into qb+1, phase-A chunks interleaved at m boundaries, so the
    in-order PE queue never head-of-line blocks on DVE/Act chains.
"""

import numpy as np

import concourse.bass as bass
import concourse.bacc as bacc
import concourse.mybir as mybir
import concourse.tile as tile
from concourse.bass import ds, ts
from concourse.bass_utils import run_bass_kernel_spmd

F32 = mybir.dt.float32
F16 = mybir.dt.float16
F8 = mybir.dt.float8e4
DR = mybir.MatmulPerfMode.DoubleRow

B, S, D, H, DK = 4, 2048, 1024, 16, 64
THETA = 10000.0
NH = 8
HD = NH * DK
P = 128
NEG = -28000.0
NEG8 = -240.0

CFG = {
    "s_dr": "ge512",   # "none" | "ge512" | "all"
    "proj_dr": False,
    "proj_hilo": True,  # hi+lo fp8e4m3 DoubleRow projections (W pre-scaled x16)
    "rope_mul_pool": False,
}
WSC = 16.0  # weight pre-scale for hi/lo fp8 (folded out via rope tables / araw)


def build_attention_nc(nrep=1):
    nc = bacc.Bacc("TRN2", target_bir_lowering=False, debug=False)

    xh = nc.dram_tensor("xh", [D // P, P, S], F16, kind="ExternalInput")
    wqh = nc.dram_tensor("wqh", [D // P, P, HD], F16, kind="ExternalInput")
    wkh = nc.dram_tensor("wkh", [D // P, P, HD], F16, kind="ExternalInput")
    wvh = nc.dram_tensor("wvh", [D // P, P, HD], F16, kind="ExternalInput")
    woh = nc.dram_tensor("woh", [HD // P, P, D], F16, kind="ExternalInput")
    cosP = nc.dram_tensor("cosP", [P, S], F16, kind="ExternalInput")
    sinP = nc.dram_tensor("sinP", [P, S], F16, kind="ExternalInput")
    ident = nc.dram_tensor("ident", [P, P], F16, kind="ExternalInput")
    umask = nc.dram_tensor("umask", [P, 512], F16, kind="ExternalInput")
    negI = nc.dram_tensor("negI", [P, P], F16, kind="ExternalInput")
    umask8 = nc.dram_tensor("umask8", [P, 2, 512], F8, kind="ExternalInput")
    negI8 = nc.dram_tensor("negI8", [P, 2, P], F8, kind="ExternalInput")
    outT = nc.dram_tensor("outT", [D, S], F32, kind="ExternalOutput")
    if CFG.get("debug"):
        dQT = nc.dram_tensor("dQT", [P, 4, S], F16, kind="ExternalOutput")
        dKT = nc.dram_tensor("dKT", [P, 4, S], F16, kind="ExternalOutput")
        dVP = nc.dram_tensor("dVP", [P, 16, NH, DK + 1], F16, kind="ExternalOutput")
        dHOPT = nc.dram_tensor("dHOPT", [P, 4, S], F16, kind="ExternalOutput")
        CFG["_dbg"] = (dQT, dKT, dVP, dHOPT)

    hl = None
    if CFG["proj_hilo"]:
        hl = {}
        for nm in ("x8h", "x8l"):
            hl[nm] = nc.dram_tensor(nm, [P, D // 256, 2, S], F8, kind="ExternalInput")
        for nm in ("wq8h", "wq8l", "wk8h", "wk8l", "wv8h", "wv8l"):
            hl[nm] = nc.dram_tensor(nm, [P, D // 256, 2, HD], F8, kind="ExternalInput")

    with tile.TileContext(nc) as tc:
        if nrep == 1:
            _attn_tile(tc, xh, hl, wqh, wkh, wvh, woh, cosP, sinP,
                       ident, umask, negI, umask8, negI8, outT)
        else:
            with tc.For_i(0, nrep, 1):
                _attn_tile(tc, xh, hl, wqh, wkh, wvh, woh, cosP,
                           sinP, ident, umask, negI, umask8, negI8, outT)
    nc.compile()
    return nc


def _attn_tile(tc, xh, hl, wqh, wkh, wvh, woh, cosP, sinP, ident,
               umask, negI, umask8, negI8, outT):
    nc = tc.nc
    s_dr = CFG["s_dr"]
    hilo = CFG["proj_hilo"]
    need_f8 = s_dr != "none"
    need_f16qt = s_dr != "all"

    with (
        tc.tile_pool(name="per", bufs=1) as per,
        tc.tile_pool(name="ab", bufs=6) as abp,
        tc.tile_pool(name="rt", bufs=4) as rtp,
        tc.tile_pool(name="ptp", bufs=3) as ptp,
        tc.tile_pool(name="arp", bufs=4) as arp,
        tc.tile_pool(name="rcp", bufs=2) as rcp,
        tc.tile_pool(name="dgp", bufs=4) as dgp,
        tc.tile_pool(name="obp", bufs=2) as obp,
        tc.tile_pool(name="psS", bufs=2, space="PSUM") as psS,
        tc.tile_pool(name="psPV", bufs=2, space="PSUM") as psPV,
        tc.tile_pool(name="psC", bufs=2, space="PSUM") as psC,
    ):
        # ---------------- persistent tiles + input DMAs ----------------
        if need_f16qt:
            QT = per.tile([P, 4, S], F16, tag="QT")
            KT = per.tile([P, 4, S], F16, tag="KT")
        else:
            QT = KT = None
        if need_f8:
            QT8 = per.tile([P, 4, S], F8, tag="QT8")
            KT8 = per.tile([P, 4, S], F8, tag="KT8")
        else:
            QT8 = KT8 = None
        VP = per.tile([P, 16, NH, DK + 1], F16, tag="VP")
        HOPT = per.tile([P, 4, S], F16, tag="HOPT")

        # ones column of VP (col 0 per head)
        nc.vector.memset(VP[:, :, :, 0:1], 1.0)

        # DMAs ordered so phase_a(0, 0) can start as early as possible:
        # j-pair-0 halves of wq first, then the first x chunk, wk, cos/sin, wv.
        if hilo:
            hsb = {}
            for nm in ("x8h", "x8l"):
                hsb[nm] = per.tile([P, 4, 2, S], F8, tag=nm, name=nm)
            for nm in ("wq8h", "wq8l", "wk8h", "wk8l", "wv8h", "wv8l"):
                hsb[nm] = per.tile([P, 4, 2, HD], F8, tag=nm, name=nm)

            def hsrc(nm):
                return hl[nm].ap()

            for nm in ("wq8h", "wq8l"):
                nc.sync.dma_start(hsb[nm][:, :, :, 0:256], hsrc(nm)[:, :, :, 0:256])
            for nm in ("x8h", "x8l"):
                nc.sync.dma_start(hsb[nm][:, :, :, 0:512], hsrc(nm)[:, :, :, 0:512])
            for nm in ("wk8h", "wk8l"):
                nc.sync.dma_start(hsb[nm][:, :, :, 0:256], hsrc(nm)[:, :, :, 0:256])
            cos_sb = per.tile([P, S], F16, tag="cos")
            sin_sb = per.tile([P, S], F16, tag="sin")
            nc.sync.dma_start(cos_sb, cosP.ap())
            nc.sync.dma_start(sin_sb, sinP.ap())
            for nm in ("wv8h", "wv8l"):
                nc.sync.dma_start(hsb[nm], hsrc(nm))
            for nm in ("wq8h", "wq8l", "wk8h", "wk8l"):
                nc.sync.dma_start(hsb[nm][:, :, :, 256:512], hsrc(nm)[:, :, :, 256:512])
            for tci in range(1, 4):
                tsl = ds(tci * 512, 512)
                for nm in ("x8h", "x8l"):
                    nc.sync.dma_start(hsb[nm][:, :, :, tsl], hsrc(nm)[:, :, :, tsl])
        else:
            xh_sb = per.tile([P, 8, S], F16, tag="xh")
            xh_src = xh.ap().rearrange("o p t -> p o t")
            wq_sb = per.tile([P, 8, HD], F16, tag="wq")
            wk_sb = per.tile([P, 8, HD], F16, tag="wk")
            wq_src = wqh.ap().rearrange("o p d -> p o d")
            wk_src = wkh.ap().rearrange("o p d -> p o d")
            for jp in (0, 1):
                dsl = ds(jp * 256, 256)
                nc.sync.dma_start(wq_sb[:, :, dsl], wq_src[:, :, dsl])
                nc.sync.dma_start(wk_sb[:, :, dsl], wk_src[:, :, dsl])
            nc.sync.dma_start(xh_sb[:, :, 0:512], xh_src[:, :, 0:512])
            cos_sb = per.tile([P, S], F16, tag="cos")
            sin_sb = per.tile([P, S], F16, tag="sin")
            nc.sync.dma_start(cos_sb, cosP.ap())
            nc.sync.dma_start(sin_sb, sinP.ap())
            wv_sb = per.tile([P, 8, HD], F16, tag="wv")
            nc.sync.dma_start(wv_sb, wvh.ap().rearrange("o p d -> p o d"))
            for tci in range(1, 4):
                tsl = ds(tci * 512, 512)
                nc.sync.dma_start(xh_sb[:, :, tsl], xh_src[:, :, tsl])
        wo_sb = per.tile([P, 4, D], F16, tag="wo")
        nc.sync.dma_start(wo_sb, woh.ap().rearrange("m p o -> p m o"))
        I_sb = per.tile([P, P], F16, tag="I")
        nc.sync.dma_start(I_sb, ident.ap())
        z_sb = per.tile([P, 512], F16, tag="z")
        nc.vector.memset(z_sb[0:1, :], 0.0)
        um_sb = per.tile([P, 512], F16, tag="um")
        nc.sync.dma_start(um_sb, umask.ap())
        nI_sb = per.tile([P, P], F16, tag="nI")
        nc.sync.dma_start(nI_sb, negI.ap())
        if need_f8:
            um8_sb = per.tile([P, 2, 512], F8, tag="um8")
            nc.sync.dma_start(um8_sb, umask8.ap())
            nI8_sb = per.tile([P, 2, P], F8, tag="nI8")
            nc.sync.dma_start(nI8_sb, negI8.ap())

        outT_ap = outT.ap()

        def act_copy(out, in_):
            nc.scalar.activation(out, in_, mybir.ActivationFunctionType.Copy)

        # ---------------- phase A half-chunk (t block 512, one j-pair) ----
        def phase_a(tci, jp):
            tsl = ds(tci * 512, 512)
            abt = []
            for wname in ("wq", "wk"):
                for j in (2 * jp, 2 * jp + 1):
                    ps = psS.tile([P, 512], F32, tag="s", name=f"pa{tci}{j}")
                    if hilo:
                        first = True
                        for c in range(4):
                            for wsfx, xsfx in (("h", "h"), ("h", "l"), ("l", "h")):
                                nc.tensor.matmul(
                                    ps,
                                    lhsT=hsb[f"{wname}8{wsfx}"][:, c, :, ts(j, P)],
                                    rhs=hsb[f"x8{xsfx}"][:, c, :, tsl],
                                    start=first, stop=(c == 3 and wsfx == "l"),
                                    perf_mode=DR,
                                )
                                first = False
                    else:
                        w = wq_sb if wname == "wq" else wk_sb
                        for o in range(8):
                            nc.tensor.matmul(
                                ps,
                                lhsT=w[:, o, ts(j, P)],
                                rhs=xh_sb[:, o, tsl],
                                start=(o == 0), stop=(o == 7),
                            )
                    ab = abp.tile([P, 512], F16, tag="ab", name=f"ab{j}")
                    nc.vector.tensor_copy(ab, ps)
                    abt.append(ab)
            # V projection for two t-subtiles
            for tt in (2 * jp, 2 * jp + 1):
                psv = psS.tile([P, 512], F32, tag="s", name=f"pvv{tci}{tt}")
                xsl = ds(tci * 512 + tt * P, P)
                if hilo:
                    first = True
                    for c in range(4):
                        for xsfx, wsfx in (("h", "h"), ("h", "l"), ("l", "h")):
                            nc.tensor.matmul(
                                psv,
                                lhsT=hsb[f"x8{xsfx}"][:, c, :, xsl],
                                rhs=hsb[f"wv8{wsfx}"][:, c, :, :],
                                start=first, stop=(c == 3 and xsfx == "l"),
                                perf_mode=DR,
                            )
                            first = False
                else:
                    for o in range(8):
                        nc.tensor.matmul(
                            psv,
                            lhsT=xh_sb[:, o, xsl],
                            rhs=wv_sb[:, o, :],
                            start=(o == 0), stop=(o == 7),
                        )
                nc.vector.tensor_copy(
                    VP[:, tci * 4 + tt, :, 1:],
                    psv.rearrange("p (h c) -> p h c", c=DK),
                )
            # RoPE for this j-pair, both tensors
            mul_eng = nc.gpsimd if CFG["rope_mul_pool"] else nc.vector
            for tensor in (0, 1):
                A, Bb = abt[2 * tensor], abt[2 * tensor + 1]
                dst = (QT, KT)[tensor]
                dst8 = (QT8, KT8)[tensor]
                je, jo = 2 * jp, 2 * jp + 1
                t1 = rtp.tile([P, 512], F16, tag="rt", name="t1")
                t2 = rtp.tile([P, 512], F16, tag="rt", name="t2")
                mul_eng.tensor_mul(t1, cos_sb[:, tsl], A)
                mul_eng.tensor_mul(t2, sin_sb[:, tsl], Bb)
                t3 = rtp.tile([P, 512], F16, tag="rt", name="t3")
                t4 = rtp.tile([P, 512], F16, tag="rt", name="t4")
                mul_eng.tensor_mul(t3, sin_sb[:, tsl], A)
                mul_eng.tensor_mul(t4, cos_sb[:, tsl], Bb)
                if need_f16qt:
                    nc.vector.tensor_sub(dst[:, je, tsl], t1, t2)
                    nc.vector.tensor_add(dst[:, jo, tsl], t3, t4)
                    if need_f8:
                        nc.gpsimd.tensor_copy(dst8[:, je, tsl], dst[:, je, tsl])
                        nc.gpsimd.tensor_copy(dst8[:, jo, tsl], dst[:, jo, tsl])
                else:
                    nc.vector.tensor_sub(dst8[:, je, tsl], t1, t2)
                    nc.vector.tensor_add(dst8[:, jo, tsl], t3, t4)

        # ---------------- attention stages ----------------
        def kt_loop(qb, m):
            """S + exp + PV accumulation for head pair m of q block qb.
            Returns state consumed by normalize()."""
            use_dr = s_dr == "all" or (s_dr == "ge512" and qb >= 1)
            pvt = [
                psPV.tile([P, 2, 130], F32, tag="pv", name=f"pv{qb}{m}{i}")
                for i in range(2)
            ]
            # start=True zeroes the whole 2KB bank (ZERO_REGION_SIZE), so a
            # shared-bank tile is zeroed once up front and all sub-region
            # accumulations use start=False.
            for i in range(2):
                nc.tensor.matmul(
                    pvt[i], lhsT=z_sb[0:1, 0:P], rhs=z_sb[0:1, 0:260],
                    start=True, stop=False, skip_group_check=True,
                )
            nkt = 4 * qb + 4
            for kt in range(nkt):
                roff = kt - 4 * qb
                c0 = P * max(roff, 0)
                live = 512 - c0
                s2 = psS.tile([P, 2, 512], F32, tag="s", name=f"s{qb}{m}{kt}")
                for e in (0, 1):
                    h = 2 * m + e
                    g, h4 = h // 4, h % 4
                    pr = ds(h4 * 32, 32)
                    tpos = (h4 * 32, 0)
                    if use_dr:
                        nc.tensor.matmul(
                            s2[:, e, c0:],
                            lhsT=KT8[pr, ds(2 * g, 2), ts(kt, P)],
                            rhs=QT8[pr, ds(2 * g, 2), ds(qb * 512 + c0, live)],
                            start=True, stop=(roff < 0),
                            perf_mode=DR,
                            skip_group_check=True,
                            tile_position=tpos,
                        )
                    else:
                        nc.tensor.matmul(
                            s2[:, e, c0:],
                            lhsT=KT[pr, 2 * g, ts(kt, P)],
                            rhs=QT[pr, 2 * g, ds(qb * 512 + c0, live)],
                            start=True, stop=False,
                            skip_group_check=True,
                            tile_position=tpos,
                        )
                        nc.tensor.matmul(
                            s2[:, e, c0:],
                            lhsT=KT[pr, 2 * g + 1, ts(kt, P)],
                            rhs=QT[pr, 2 * g + 1, ds(qb * 512 + c0, live)],
                            start=False, stop=(roff < 0),
                            skip_group_check=True,
                            tile_position=tpos,
                        )
                    if roff >= 0:
                        if use_dr:
                            nc.tensor.matmul(
                                s2[:, e, ds(c0, P)],
                                lhsT=nI8_sb,
                                rhs=um8_sb[:, :, 0:P],
                                start=False, stop=True,
                                perf_mode=DR,
                                skip_group_check=True,
                            )
                        else:
                            nc.tensor.matmul(
                                s2[:, e, ds(c0, P)],
                                lhsT=nI_sb,
                                rhs=um_sb[:, 0:P],
                                start=False, stop=True,
                                skip_group_check=True,
                            )
                pt2 = ptp.tile([P, 2, 512], F16, tag="pt")
                nc.scalar.activation(
                    pt2[:, :, c0:], s2[:, :, c0:],
                    mybir.ActivationFunctionType.Exp, scale=0.125,
                )
                for e in (0, 1):
                    h = 2 * m + e
                    for qt in range(4):
                        if qt < roff:
                            continue
                        nc.tensor.matmul(
                            pvt[qt // 2][:, qt % 2, ds(e * 65, 65)],
                            lhsT=pt2[:, e, ds(qt * P, P)],
                            rhs=VP[:, kt, h, :],
                            start=False,
                            stop=(kt == 4 * qb + qt),
                            skip_group_check=True,
                        )
            return pvt

        def norm_a(qb, m, pvt):
            """Drain pv psum right away: recip denominators to SBUF + raw
            fp16 copy. DVE-only, so the PE queue is not blocked; frees the
            pv psum ring for the next kt_loop."""
            rc = rcp.tile([P, 2, 2, 2], F32, tag="rc", name=f"rc{qb}{m}")
            araw = []
            for i in (0, 1):
                rsl = pvt[i].rearrange("p a (b c) -> p a b c", c=65)[:, :, :, 0]
                nc.vector.reciprocal(rc[:, i], rsl)
                ar = arp.tile([P, 2, 130], F16, tag="ar", name=f"ar{i}")
                if hilo:
                    nc.vector.tensor_scalar_mul(ar, pvt[i], 1.0 / WSC)
                else:
                    nc.vector.tensor_copy(ar, pvt[i])
                araw.append(ar)
            return rc, araw

        def norm_b(qb, m, rc, araw):
            """Deferred: diag builds + scaled transpose into HOPT."""
            qsl = ds(qb * 512, 512)
            tp = psC.tile([P, 512], F32, tag="c", name=f"tp{qb}{m}")
            nc.tensor.matmul(
                tp, lhsT=z_sb[0:1, 0:P], rhs=z_sb[0:1, :],
                start=True, stop=False, skip_group_check=True,
            )
            for qt in range(4):
                for e in (0, 1):
                    dg = dgp.tile([P, P], F16, tag="dg")
                    nc.vector.tensor_scalar_mul(
                        dg, I_sb, rc[:, qt // 2, qt % 2, ds(e, 1)]
                    )
                    nc.tensor.matmul(
                        tp[ds(e * DK, DK), ts(qt, P)],
                        lhsT=araw[qt // 2][:, qt % 2, ds(e * 65 + 1, DK)],
                        rhs=dg,
                        start=False, stop=(qt == 3),
                        skip_group_check=True,
                    )
            nc.vector.tensor_copy(HOPT[:, m, qsl], tp)

        def o_proj(qb):
            qsl = ds(qb * 512, 512)
            for og in range(2):
                ob = obp.tile([P, 4, 512], F32, tag="ob")
                for oi in range(4):
                    ot = og * 4 + oi
                    psc = psC.tile([P, 512], F32, tag="c", name=f"oc{qb}{ot}")
                    for mm in range(4):
                        nc.tensor.matmul(
                            psc,
                            lhsT=wo_sb[:, mm, ts(ot, P)],
                            rhs=HOPT[:, mm, qsl],
                            start=(mm == 0), stop=(mm == 3),
                        )
                    nc.vector.tensor_copy(ob[:, oi, :], psc)
                nc.sync.dma_start(
                    outT_ap[ds(og * 512, 512), qsl].rearrange(
                        "(o p) q -> p o q", p=P
                    ),
                    ob,
                )

        # ---------------- emission schedule (software-pipelined) ----------
        # pending normalize/o_proj work is emitted one stage later so the
        # in-order PE queue always has ready S/PV work at its head.
        phase_a(0, 0)
        phase_a(0, 1)

        pend_norm = None   # (qb, m, rc, araw)
        pend_oproj = None  # qb

        # extra phase-A half-chunks interleaved at (qb, m) boundaries
        a_sched = {
            (0, 0): (1, 0), (0, 1): (1, 1),
            (0, 2): (2, 0), (0, 3): (2, 1),
            (1, 0): (3, 0), (1, 1): (3, 1),
        }

        for qb in range(4):
            for m in range(4):
                pvt = kt_loop(qb, m)
                rc, araw = norm_a(qb, m, pvt)
                if pend_norm is not None:
                    norm_b(*pend_norm)
                pend_norm = (qb, m, rc, araw)
                if (qb, m) in a_sched:
                    phase_a(*a_sched[(qb, m)])
                if pend_oproj is not None and m == 1:
                    o_proj(pend_oproj)
                    pend_oproj = None
            pend_oproj = qb
        norm_b(*pend_norm)
        o_proj(3)

        if CFG.get("debug"):
            dQT, dKT, dVP, dHOPT = CFG["_dbg"]
            nc.sync.dma_start(dQT.ap(), QT if need_f16qt else QT8)
            nc.sync.dma_start(dKT.ap(), KT if need_f16qt else KT8)
            nc.sync.dma_start(dHOPT.ap(), HOPT)
            nc.sync.dma_start(dVP.ap(), VP)


# ---------------- host side ----------------

def _qk_perm():
    perm = []
    for j in range(4):
        for h4 in range(4):
            h = (j // 2) * 4 + h4
            par = j % 2
            perm.extend(h * 64 + 2 * np.arange(32) + par)
    return np.array(perm)


_PERM = _qk_perm()


def _host_tables():
    import ml_dtypes

    F16n = np.float16
    i = np.arange(32, dtype=np.float32)
    inv_freq = (THETA ** (2.0 * i / DK)).astype(np.float32)
    t = np.arange(S, dtype=np.float32)
    ang = t[None, :] / inv_freq[:, None]          # [32, S]
    sc = 1.0 / WSC if CFG["proj_hilo"] else 1.0
    cosP = np.tile(sc * np.cos(ang), (4, 1)).astype(F16n)   # [128, S]
    sinP = np.tile(sc * np.sin(ang), (4, 1)).astype(F16n)
    ident = np.eye(P, dtype=F16n)
    kk = np.arange(P)[:, None]
    qq = np.arange(512)[None, :]
    umask = (kk > qq).astype(F16n)                # [128, 512]
    negI = (NEG * np.eye(P)).astype(F16n)
    F8n = ml_dtypes.float8_e4m3fn
    negI8 = np.zeros((P, 2, P), dtype=np.float32)
    umask8 = np.zeros((P, 2, 512), dtype=np.float32)
    for ii in range(2):
        for p in range(64):
            negI8[p, ii, ii * 64 + p] = NEG8
            umask8[p, ii, :] = ((ii * 64 + p) > qq[0]).astype(np.float32)
    return cosP, sinP, ident, umask, negI, umask8.astype(F8n), negI8.astype(F8n)


def make_in_maps(x, Wq, Wk, Wv, Wo):
    import ml_dtypes

    F16n = np.float16
    F8n = ml_dtypes.float8_e4m3fn
    cosP, sinP, ident, umask, negI, umask8, negI8 = _host_tables()
    in_maps = []
    for c in range(8):
        b, hh = c // 2, c % 2
        sl = slice(hh * HD, (hh + 1) * HD)
        xT = np.ascontiguousarray(x[b].T)                      # [1024, 2048]
        wq = Wq[sl, :][_PERM]
        wk = Wk[sl, :][_PERM]
        wv = Wv[sl, :]
        wo = Wo[:, sl]                                         # [1024, 512]
        woh = np.ascontiguousarray(wo.T).reshape(4, P, D)
        m = {
            "xh": xT.reshape(8, P, S).astype(F16n),
            "wqh": np.ascontiguousarray(wq.T).reshape(8, P, HD).astype(F16n),
            "wkh": np.ascontiguousarray(wk.T).reshape(8, P, HD).astype(F16n),
            "wvh": np.ascontiguousarray(wv.T).reshape(8, P, HD).astype(F16n),
            "woh": woh.astype(F16n),
            "cosP": cosP, "sinP": sinP, "ident": ident,
            "umask": umask, "negI": negI, "umask8": umask8, "negI8": negI8,
        }
        if CFG["proj_hilo"]:
            def drfmt(a, n):
                return np.ascontiguousarray(a).reshape(4, 2, P, n).transpose(2, 0, 1, 3)

            def hilo8(a, n, scale):
                a = drfmt(a, n) * scale
                hi = a.astype(F8n)
                lo = (a - hi.astype(np.float32)).astype(F8n)
                return hi, lo

            m["x8h"], m["x8l"] = hilo8(xT, S, 1.0)
            m["wq8h"], m["wq8l"] = hilo8(wq.T, HD, WSC)
            m["wk8h"], m["wk8l"] = hilo8(wk.T, HD, WSC)
            m["wv8h"], m["wv8l"] = hilo8(wv.T, HD, WSC)
        in_maps.append(m)
    return in_maps


def gather_out(core_outs):
    out = np.empty((B, S, D), dtype=np.float32)
    for b in range(B):
        out[b] = (core_outs[2 * b]["outT"] + core_outs[2 * b + 1]["outT"]).T
    return out


_NC_CACHE = {}


def kernel(x, Wq, Wk, Wv, Wo):
    x = np.asarray(x, dtype=np.float32)
    Wq = np.asarray(Wq, dtype=np.float32)
    Wk = np.asarray(Wk, dtype=np.float32)
    Wv = np.asarray(Wv, dtype=np.float32)
    Wo = np.asarray(Wo, dtype=np.float32)
    if "nc" not in _NC_CACHE:
        _NC_CACHE["nc"] = build_attention_nc()
    nc = _NC_CACHE["nc"]
    in_maps = make_in_maps(x, Wq, Wk, Wv, Wo)
    res = run_bass_kernel_spmd(nc, in_maps, core_ids=list(range(8)))
    return gather_out(res.results)


# revision 4
# speedup vs baseline: 1.1379x; 1.0057x over previous
"""Causal MHA (B=4, S=2048, D=1024, H=16, RoPE) on 8 trn2 cores — v2.

Sharding: core c -> batch c//2, head-half c%2 (8 heads / 512 dims per core).

Design vs v1 baseline:
  - Q/K weights host-permuted into even/odd 32-blocks per 4-head group so
    RoPE needs NO partition-swap DMA: psum tile pairs (j, j+1) hold the
    even/odd halves at identical partition indices and the rotation is plain
    elementwise tensor ops (fp16, 2x DVE rate).
  - fp16 downstream (P, V, attn, Wo): 1 cyc/row matmuls.
  - Optional fp8e4m3 DoubleRow scores (0.5 cyc/row): the even/odd layout is
    natively DR-compatible ([32 parts, 2 j-slots, t] APs), no re-layout DMA.
  - PV in [q, dv] orientation (moving dim 65): psum accumulators
    [128, 2qt, 130] with ones-column densities; normalization via
    per-partition recips + diag-matmul transpose back to [d', t].
  - Software-pipelined emission: normalize(m) deferred into (m+1)'s kt loop,
    O-proj(qb) into qb+1, phase-A chunks interleaved at m boundaries, so the
    in-order PE queue never head-of-line blocks on DVE/Act chains.
"""

import numpy as np

import concourse.bass as bass
import concourse.bacc as bacc
import concourse.mybir as mybir
import concourse.tile as tile
from concourse.bass import ds, ts
from concourse.bass_utils import run_bass_kernel_spmd

F32 = mybir.dt.float32
F16 = mybir.dt.float16
F8 = mybir.dt.float8e4
DR = mybir.MatmulPerfMode.DoubleRow

B, S, D, H, DK = 4, 2048, 1024, 16, 64
THETA = 10000.0
NH = 8
HD = NH * DK
P = 128
NEG = -28000.0
NEG8 = -240.0

CFG = {
    "s_dr": "ge512",   # "none" | "ge512" | "all"
    "proj_dr": False,
    "proj_hilo": True,  # hi+lo fp8e4m3 DoubleRow projections (W pre-scaled x16)
    "rope_mul_pool": False,
}
WSC = 16.0  # weight pre-scale for hi/lo fp8 (folded out via rope tables / araw)


def build_attention_nc(nrep=1):
    nc = bacc.Bacc("TRN2", target_bir_lowering=False, debug=False)

    xh = nc.dram_tensor("xh", [D // P, P, S], F16, kind="ExternalInput")
    wqh = nc.dram_tensor("wqh", [D // P, P, HD], F16, kind="ExternalInput")
    wkh = nc.dram_tensor("wkh", [D // P, P, HD], F16, kind="ExternalInput")
    wvh = nc.dram_tensor("wvh", [D // P, P, HD], F16, kind="ExternalInput")
    woh = nc.dram_tensor("woh", [HD // P, P, D], F16, kind="ExternalInput")
    cosP = nc.dram_tensor("cosP", [P, S], F16, kind="ExternalInput")
    sinP = nc.dram_tensor("sinP", [P, S], F16, kind="ExternalInput")
    ident = nc.dram_tensor("ident", [P, P], F16, kind="ExternalInput")
    umask = nc.dram_tensor("umask", [P, 512], F16, kind="ExternalInput")
    negI = nc.dram_tensor("negI", [P, P], F16, kind="ExternalInput")
    umask8 = nc.dram_tensor("umask8", [P, 2, 512], F8, kind="ExternalInput")
    negI8 = nc.dram_tensor("negI8", [P, 2, P], F8, kind="ExternalInput")
    outT = nc.dram_tensor("outT", [D, S], F32, kind="ExternalOutput")
    if CFG.get("debug"):
        dQT = nc.dram_tensor("dQT", [P, 4, S], F16, kind="ExternalOutput")
        dKT = nc.dram_tensor("dKT", [P, 4, S], F16, kind="ExternalOutput")
        dVP = nc.dram_tensor("dVP", [P, 16, NH, DK + 1], F16, kind="ExternalOutput")
        dHOPT = nc.dram_tensor("dHOPT", [P, 4, S], F16, kind="ExternalOutput")
        CFG["_dbg"] = (dQT, dKT, dVP, dHOPT)

    hl = None
    if CFG["proj_hilo"]:
        hl = {}
        for nm in ("x8h", "x8l"):
            hl[nm] = nc.dram_tensor(nm, [P, D // 256, 2, S], F8, kind="ExternalInput")
        for nm in ("wq8h", "wq8l", "wk8h", "wk8l", "wv8h", "wv8l"):
            hl[nm] = nc.dram_tensor(nm, [P, D // 256, 2, HD], F8, kind="ExternalInput")

    with tile.TileContext(nc) as tc:
        if nrep == 1:
            _attn_tile(tc, xh, hl, wqh, wkh, wvh, woh, cosP, sinP,
                       ident, umask, negI, umask8, negI8, outT)
        else:
            with tc.For_i(0, nrep, 1):
                _attn_tile(tc, xh, hl, wqh, wkh, wvh, woh, cosP,
                           sinP, ident, umask, negI, umask8, negI8, outT)
    nc.compile()
    return nc


def _attn_tile(tc, xh, hl, wqh, wkh, wvh, woh, cosP, sinP, ident,
               umask, negI, umask8, negI8, outT):
    nc = tc.nc
    s_dr = CFG["s_dr"]
    hilo = CFG["proj_hilo"]
    need_f8 = s_dr != "none"
    need_f16qt = s_dr != "all"

    with (
        tc.tile_pool(name="per", bufs=1) as per,
        tc.tile_pool(name="ab", bufs=6) as abp,
        tc.tile_pool(name="rt", bufs=4) as rtp,
        tc.tile_pool(name="ptp", bufs=3) as ptp,
        tc.tile_pool(name="arp", bufs=4) as arp,
        tc.tile_pool(name="rcp", bufs=2) as rcp,
        tc.tile_pool(name="dgp", bufs=4) as dgp,
        tc.tile_pool(name="obp", bufs=2) as obp,
        tc.tile_pool(name="psS", bufs=2, space="PSUM") as psS,
        tc.tile_pool(name="psPV", bufs=2, space="PSUM") as psPV,
        tc.tile_pool(name="psC", bufs=2, space="PSUM") as psC,
    ):
        # ---------------- persistent tiles + input DMAs ----------------
        if need_f16qt:
            QT = per.tile([P, 4, S], F16, tag="QT")
            KT = per.tile([P, 4, S], F16, tag="KT")
        else:
            QT = KT = None
        if need_f8:
            QT8 = per.tile([P, 4, S], F8, tag="QT8")
            KT8 = per.tile([P, 4, S], F8, tag="KT8")
        else:
            QT8 = KT8 = None
        VP = per.tile([P, 16, NH, DK + 1], F16, tag="VP")
        HOPT = per.tile([P, 4, S], F16, tag="HOPT")

        # ones column of VP (col 0 per head)
        nc.vector.memset(VP[:, :, :, 0:1], 1.0)

        # DMAs ordered so phase_a(0, 0) can start as early as possible:
        # j-pair-0 halves of wq first, then the first x chunk, wk, cos/sin, wv.
        if hilo:
            hsb = {}
            for nm in ("x8h", "x8l"):
                hsb[nm] = per.tile([P, 4, 2, S], F8, tag=nm, name=nm)
            for nm in ("wq8h", "wq8l", "wk8h", "wk8l", "wv8h", "wv8l"):
                hsb[nm] = per.tile([P, 4, 2, HD], F8, tag=nm, name=nm)

            def hsrc(nm):
                return hl[nm].ap()

            for nm in ("wq8h", "wq8l"):
                nc.sync.dma_start(hsb[nm][:, :, :, 0:256], hsrc(nm)[:, :, :, 0:256])
            for nm in ("x8h", "x8l"):
                nc.sync.dma_start(hsb[nm][:, :, :, 0:512], hsrc(nm)[:, :, :, 0:512])
            for nm in ("wk8h", "wk8l"):
                nc.sync.dma_start(hsb[nm][:, :, :, 0:256], hsrc(nm)[:, :, :, 0:256])
            for nm in ("wv8h", "wv8l"):
                nc.sync.dma_start(hsb[nm][:, :, 0:1, :], hsrc(nm)[:, :, 0:1, :])
            cos_sb = per.tile([P, S], F16, tag="cos")
            sin_sb = per.tile([P, S], F16, tag="sin")
            nc.sync.dma_start(cos_sb, cosP.ap())
            nc.sync.dma_start(sin_sb, sinP.ap())
            for nm in ("wv8h", "wv8l"):
                nc.sync.dma_start(hsb[nm][:, :, 1:2, :], hsrc(nm)[:, :, 1:2, :])
            for nm in ("wq8h", "wq8l", "wk8h", "wk8l"):
                nc.sync.dma_start(hsb[nm][:, :, :, 256:512], hsrc(nm)[:, :, :, 256:512])
            for tci in range(1, 4):
                tsl = ds(tci * 512, 512)
                for nm in ("x8h", "x8l"):
                    nc.sync.dma_start(hsb[nm][:, :, :, tsl], hsrc(nm)[:, :, :, tsl])
        else:
            xh_sb = per.tile([P, 8, S], F16, tag="xh")
            xh_src = xh.ap().rearrange("o p t -> p o t")
            wq_sb = per.tile([P, 8, HD], F16, tag="wq")
            wk_sb = per.tile([P, 8, HD], F16, tag="wk")
            wq_src = wqh.ap().rearrange("o p d -> p o d")
            wk_src = wkh.ap().rearrange("o p d -> p o d")
            for jp in (0, 1):
                dsl = ds(jp * 256, 256)
                nc.sync.dma_start(wq_sb[:, :, dsl], wq_src[:, :, dsl])
                nc.sync.dma_start(wk_sb[:, :, dsl], wk_src[:, :, dsl])
            nc.sync.dma_start(xh_sb[:, :, 0:512], xh_src[:, :, 0:512])
            cos_sb = per.tile([P, S], F16, tag="cos")
            sin_sb = per.tile([P, S], F16, tag="sin")
            nc.sync.dma_start(cos_sb, cosP.ap())
            nc.sync.dma_start(sin_sb, sinP.ap())
            wv_sb = per.tile([P, 8, HD], F16, tag="wv")
            nc.sync.dma_start(wv_sb, wvh.ap().rearrange("o p d -> p o d"))
            for tci in range(1, 4):
                tsl = ds(tci * 512, 512)
                nc.sync.dma_start(xh_sb[:, :, tsl], xh_src[:, :, tsl])
        wo_sb = per.tile([P, 4, D], F16, tag="wo")
        nc.sync.dma_start(wo_sb, woh.ap().rearrange("m p o -> p m o"))
        I_sb = per.tile([P, P], F16, tag="I")
        nc.sync.dma_start(I_sb, ident.ap())
        z_sb = per.tile([P, 512], F16, tag="z")
        nc.vector.memset(z_sb[0:1, :], 0.0)
        um_sb = per.tile([P, 512], F16, tag="um")
        nc.sync.dma_start(um_sb, umask.ap())
        nI_sb = per.tile([P, P], F16, tag="nI")
        nc.sync.dma_start(nI_sb, negI.ap())
        if need_f8:
            um8_sb = per.tile([P, 2, 512], F8, tag="um8")
            nc.sync.dma_start(um8_sb, umask8.ap())
            nI8_sb = per.tile([P, 2, P], F8, tag="nI8")
            nc.sync.dma_start(nI8_sb, negI8.ap())

        outT_ap = outT.ap()

        def act_copy(out, in_):
            nc.scalar.activation(out, in_, mybir.ActivationFunctionType.Copy)

        # ---------------- phase A half-chunk (t block 512, one j-pair) ----
        def phase_a_units(tci, jp):
            """Yield small closures: 2 Q-proj tiles, ropeQ, 2 K-proj tiles,
            ropeK, 2 V tiles. Emitted piecemeal between attention kts."""
            tsl = ds(tci * 512, 512)
            abt = {}

            def proj_unit(wname, j):
              def go():
                    ps = psC.tile([P, 512], F32, tag="c", name=f"pa{tci}{j}")
                    if hilo:
                        first = True
                        for c in range(4):
                            for wsfx, xsfx in (("h", "h"), ("h", "l"), ("l", "h")):
                                nc.tensor.matmul(
                                    ps,
                                    lhsT=hsb[f"{wname}8{wsfx}"][:, c, :, ts(j, P)],
                                    rhs=hsb[f"x8{xsfx}"][:, c, :, tsl],
                                    start=first, stop=(c == 3 and wsfx == "l"),
                                    perf_mode=DR,
                                )
                                first = False
                    else:
                        w = wq_sb if wname == "wq" else wk_sb
                        for o in range(8):
                            nc.tensor.matmul(
                                ps,
                                lhsT=w[:, o, ts(j, P)],
                                rhs=xh_sb[:, o, tsl],
                                start=(o == 0), stop=(o == 7),
                            )
                    ab = abp.tile([P, 512], F16, tag="ab", name=f"ab{j}")
                    nc.vector.tensor_copy(ab, ps)
                    abt[j] = ab
              return go

            def v_unit(tt):
              def go():
                    psv = psC.tile([P, 512], F32, tag="c", name=f"pvv{tci}{tt}")
                    xsl = ds(tci * 512 + tt * P, P)
                    if hilo:
                        first = True
                        for c in range(4):
                            for xsfx, wsfx in (("h", "h"), ("h", "l"), ("l", "h")):
                                nc.tensor.matmul(
                                    psv,
                                    lhsT=hsb[f"x8{xsfx}"][:, c, :, xsl],
                                    rhs=hsb[f"wv8{wsfx}"][:, c, :, :],
                                    start=first, stop=(c == 3 and xsfx == "l"),
                                    perf_mode=DR,
                                )
                                first = False
                    else:
                        for o in range(8):
                            nc.tensor.matmul(
                                psv,
                                lhsT=xh_sb[:, o, xsl],
                                rhs=wv_sb[:, o, :],
                                start=(o == 0), stop=(o == 7),
                            )
                    nc.vector.tensor_copy(
                        VP[:, tci * 4 + tt, :, 1:],
                        psv.rearrange("p (h c) -> p h c", c=DK),
                    )
              return go

            def rope_unit(tensor):
              def go():
                    mul_eng = nc.gpsimd if CFG["rope_mul_pool"] else nc.vector
                    je, jo = 2 * jp, 2 * jp + 1
                    A, Bb = abt[je] if tensor == 0 else abt[je + 10], None
                    # resolved below
              return go

            yield proj_unit("wq", 2 * jp)
            yield proj_unit("wq", 2 * jp + 1)
            yield _rope_closure(tci, jp, abt, 0)
            yield proj_unit("wk", 2 * jp)
            yield proj_unit("wk", 2 * jp + 1)
            yield _rope_closure(tci, jp, abt, 1)
            yield v_unit(2 * jp)
            yield v_unit(2 * jp + 1)

        def _unused(tci, jp):
            tsl = ds(tci * 512, 512)
            abt = []
            for wname in ("wq", "wk"):
                for j in (2 * jp, 2 * jp + 1):
                    ps = psC.tile([P, 512], F32, tag="c", name=f"pa{tci}{j}")
                    if hilo:
                        first = True
                        for c in range(4):
                            for wsfx, xsfx in (("h", "h"), ("h", "l"), ("l", "h")):
                                nc.tensor.matmul(
                                    ps,
                                    lhsT=hsb[f"{wname}8{wsfx}"][:, c, :, ts(j, P)],
                                    rhs=hsb[f"x8{xsfx}"][:, c, :, tsl],
                                    start=first, stop=(c == 3 and wsfx == "l"),
                                    perf_mode=DR,
                                )
                                first = False
                    else:
                        w = wq_sb if wname == "wq" else wk_sb
                        for o in range(8):
                            nc.tensor.matmul(
                                ps,
                                lhsT=w[:, o, ts(j, P)],
                                rhs=xh_sb[:, o, tsl],
                                start=(o == 0), stop=(o == 7),
                            )
                    ab = abp.tile([P, 512], F16, tag="ab", name=f"ab{j}")
                    act_copy(ab, ps)
                    abt.append(ab)
            # V projection for two t-subtiles
            for tt in (2 * jp, 2 * jp + 1):
                psv = psC.tile([P, 512], F32, tag="c", name=f"pvv{tci}{tt}")
                xsl = ds(tci * 512 + tt * P, P)
                if hilo:
                    first = True
                    for c in range(4):
                        for xsfx, wsfx in (("h", "h"), ("h", "l"), ("l", "h")):
                            nc.tensor.matmul(
                                psv,
                                lhsT=hsb[f"x8{xsfx}"][:, c, :, xsl],
                                rhs=hsb[f"wv8{wsfx}"][:, c, :, :],
                                start=first, stop=(c == 3 and xsfx == "l"),
                                perf_mode=DR,
                            )
                            first = False
                else:
                    for o in range(8):
                        nc.tensor.matmul(
                            psv,
                            lhsT=xh_sb[:, o, xsl],
                            rhs=wv_sb[:, o, :],
                            start=(o == 0), stop=(o == 7),
                        )
                act_copy(
                    VP[:, tci * 4 + tt, :, 1:],
                    psv.rearrange("p (h c) -> p h c", c=DK),
                )
            # RoPE for this j-pair, both tensors
            mul_eng = nc.gpsimd if CFG["rope_mul_pool"] else nc.vector
            for tensor in (0, 1):
                A, Bb = abt[2 * tensor], abt[2 * tensor + 1]
                dst = (QT, KT)[tensor]
                dst8 = (QT8, KT8)[tensor]
                je, jo = 2 * jp, 2 * jp + 1
                t1 = rtp.tile([P, 512], F16, tag="rt", name="t1")
                t2 = rtp.tile([P, 512], F16, tag="rt", name="t2")
                mul_eng.tensor_mul(t1, cos_sb[:, tsl], A)
                mul_eng.tensor_mul(t2, sin_sb[:, tsl], Bb)
                t3 = rtp.tile([P, 512], F16, tag="rt", name="t3")
                t4 = rtp.tile([P, 512], F16, tag="rt", name="t4")
                mul_eng.tensor_mul(t3, sin_sb[:, tsl], A)
                mul_eng.tensor_mul(t4, cos_sb[:, tsl], Bb)
                if need_f16qt:
                    nc.vector.tensor_sub(dst[:, je, tsl], t1, t2)
                    nc.vector.tensor_add(dst[:, jo, tsl], t3, t4)
                    if need_f8:
                        nc.gpsimd.tensor_copy(dst8[:, je, tsl], dst[:, je, tsl])
                        nc.gpsimd.tensor_copy(dst8[:, jo, tsl], dst[:, jo, tsl])
                else:
                    nc.vector.tensor_sub(dst8[:, je, tsl], t1, t2)
                    nc.vector.tensor_add(dst8[:, jo, tsl], t3, t4)

        # ---------------- attention stages ----------------
        def kt_loop(qb, m):
            """S + exp + PV accumulation for head pair m of q block qb.
            Returns state consumed by normalize()."""
            use_dr = s_dr == "all" or (s_dr == "ge512" and qb >= 1)
            pvt = [
                psPV.tile([P, 2, 130], F32, tag="pv", name=f"pv{qb}{m}{i}")
                for i in range(2)
            ]
            # start=True zeroes the whole 2KB bank (ZERO_REGION_SIZE), so a
            # shared-bank tile is zeroed once up front and all sub-region
            # accumulations use start=False.
            for i in range(2):
                nc.tensor.matmul(
                    pvt[i], lhsT=z_sb[0:1, 0:P], rhs=z_sb[0:1, 0:260],
                    start=True, stop=False, skip_group_check=True,
                )
            nkt = 4 * qb + 4
            pend_pv = None  # (kt, pt2): PV deferred one kt so S(kt+1)
                            # precedes PV(kt) in the in-order PE queue

            def emit_pv(kt, pt2):
                roff_ = kt - 4 * qb
                for e in (0, 1):
                    h = 2 * m + e
                    for qt in range(4):
                        if qt < roff_:
                            continue
                        nc.tensor.matmul(
                            pvt[qt // 2][:, qt % 2, ds(e * 65, 65)],
                            lhsT=pt2[:, e, ds(qt * P, P)],
                            rhs=VP[:, kt, h, :],
                            start=False,
                            stop=(kt == 4 * qb + qt),
                            skip_group_check=True,
                        )

            for kt in range(nkt):
                roff = kt - 4 * qb
                c0 = P * max(roff, 0)
                live = 512 - c0
                s2 = psS.tile([P, 2, 512], F32, tag="s", name=f"s{qb}{m}{kt}")
                for e in (0, 1):
                    h = 2 * m + e
                    g, h4 = h // 4, h % 4
                    pr = ds(h4 * 32, 32)
                    tpos = (h4 * 32, 0)
                    if use_dr:
                        nc.tensor.matmul(
                            s2[:, e, c0:],
                            lhsT=KT8[pr, ds(2 * g, 2), ts(kt, P)],
                            rhs=QT8[pr, ds(2 * g, 2), ds(qb * 512 + c0, live)],
                            start=True, stop=(roff < 0),
                            perf_mode=DR,
                            skip_group_check=True,
                            tile_position=tpos,
                        )
                    else:
                        nc.tensor.matmul(
                            s2[:, e, c0:],
                            lhsT=KT[pr, 2 * g, ts(kt, P)],
                            rhs=QT[pr, 2 * g, ds(qb * 512 + c0, live)],
                            start=True, stop=False,
                            skip_group_check=True,
                            tile_position=tpos,
                        )
                        nc.tensor.matmul(
                            s2[:, e, c0:],
                            lhsT=KT[pr, 2 * g + 1, ts(kt, P)],
                            rhs=QT[pr, 2 * g + 1, ds(qb * 512 + c0, live)],
                            start=False, stop=(roff < 0),
                            skip_group_check=True,
                            tile_position=tpos,
                        )
                    if roff >= 0:
                        if use_dr:
                            nc.tensor.matmul(
                                s2[:, e, ds(c0, P)],
                                lhsT=nI8_sb,
                                rhs=um8_sb[:, :, 0:P],
                                start=False, stop=True,
                                perf_mode=DR,
                                skip_group_check=True,
                            )
                        else:
                            nc.tensor.matmul(
                                s2[:, e, ds(c0, P)],
                                lhsT=nI_sb,
                                rhs=um_sb[:, 0:P],
                                start=False, stop=True,
                                skip_group_check=True,
                            )
                pt2 = ptp.tile([P, 2, 512], F16, tag="pt")
                nc.scalar.activation(
                    pt2[:, :, c0:], s2[:, :, c0:],
                    mybir.ActivationFunctionType.Exp, scale=0.125,
                )
                if pend_pv is not None:
                    emit_pv(*pend_pv)
                pend_pv = (kt, pt2)
                drain(1)
            emit_pv(*pend_pv)
            return pvt

        def norm_a(qb, m, pvt):
            """Drain pv psum right away: recip denominators to SBUF + raw
            fp16 copy. DVE-only, so the PE queue is not blocked; frees the
            pv psum ring for the next kt_loop."""
            rc = rcp.tile([P, 2, 2, 2], F32, tag="rc", name=f"rc{qb}{m}")
            araw = []
            for i in (0, 1):
                rsl = pvt[i].rearrange("p a (b c) -> p a b c", c=65)[:, :, :, 0]
                nc.vector.reciprocal(rc[:, i], rsl)
                ar = arp.tile([P, 2, 130], F16, tag="ar", name=f"ar{i}")
                if hilo:
                    nc.vector.tensor_scalar_mul(ar, pvt[i], 1.0 / WSC)
                else:
                    nc.vector.tensor_copy(ar, pvt[i])
                araw.append(ar)
            return rc, araw

        def norm_b_units(qb, m, rc, araw):
            """Deferred: diag builds + scaled transpose into HOPT, split into
            two filler units so the PE queue never blocks long on them."""
            qsl = ds(qb * 512, 512)
            st = {}

            def half(which):
                def go():
                    if which == 0:
                        st["tp"] = psC.tile([P, 512], F32, tag="c",
                                            name=f"tp{qb}{m}")
                        nc.tensor.matmul(
                            st["tp"], lhsT=z_sb[0:1, 0:P], rhs=z_sb[0:1, :],
                            start=True, stop=False, skip_group_check=True,
                        )
                    tp = st["tp"]
                    for qt in (0, 1) if which == 0 else (2, 3):
                        for e in (0, 1):
                            dg = dgp.tile([P, P], F16, tag="dg")
                            nc.vector.tensor_scalar_mul(
                                dg, I_sb, rc[:, qt // 2, qt % 2, ds(e, 1)]
                            )
                            nc.tensor.matmul(
                                tp[ds(e * DK, DK), ts(qt, P)],
                                lhsT=araw[qt // 2][:, qt % 2, ds(e * 65 + 1, DK)],
                                rhs=dg,
                                start=False, stop=(qt == 3),
                                skip_group_check=True,
                            )
                    if which == 1:
                        nc.vector.tensor_copy(HOPT[:, m, qsl], tp)
                return go

            return [half(0), half(1)]

        def norm_b(qb, m, rc, araw):
            for u in norm_b_units(qb, m, rc, araw):
                u()

        def o_proj(qb):
            qsl = ds(qb * 512, 512)
            for og in range(2):
                ob = obp.tile([P, 4, 512], F32, tag="ob")
                for oi in range(4):
                    ot = og * 4 + oi
                    psc = psC.tile([P, 512], F32, tag="c", name=f"oc{qb}{ot}")
                    for mm in range(4):
                        nc.tensor.matmul(
                            psc,
                            lhsT=wo_sb[:, mm, ts(ot, P)],
                            rhs=HOPT[:, mm, qsl],
                            start=(mm == 0), stop=(mm == 3),
                        )
                    nc.vector.tensor_copy(ob[:, oi, :], psc)
                nc.sync.dma_start(
                    outT_ap[ds(og * 512, 512), qsl].rearrange(
                        "(o p) q -> p o q", p=P
                    ),
                    ob,
                )

        # ---------------- emission schedule (software-pipelined) ----------
        # Filler units (phase-A pieces, o_proj pieces) are drained one per kt
        # inside the attention loops so the PE always interleaves projection
        # work with the exp-feeding S-matmuls instead of blocking Act behind
        # multi-microsecond projection bursts.
        import os
        from collections import deque

        filler = deque()  # (tag, closure); tag = ("a", tci) or ("o", qb)

        def drain(n=1):
            for _ in range(n):
                if filler:
                    filler.popleft()[1]()

        def flush_a(tci):
            while any(t[0] == "a" and t[1] <= tci for t, _ in filler):
                filler.popleft()[1]()

        def o_units(qb):
            qsl = ds(qb * 512, 512)
            units = []
            obt = {}

            def ou(og, oi):
                def go():
                    if oi == 0:
                        obt[og] = obp.tile([P, 4, 512], F32, tag="ob",
                                           name=f"ob{qb}{og}")
                    ot = og * 4 + oi
                    psc = psC.tile([P, 512], F32, tag="c", name=f"oc{qb}{ot}")
                    for mm in range(4):
                        nc.tensor.matmul(
                            psc,
                            lhsT=wo_sb[:, mm, ts(ot, P)],
                            rhs=HOPT[:, mm, qsl],
                            start=(mm == 0), stop=(mm == 3),
                        )
                    nc.vector.tensor_copy(obt[og][:, oi, :], psc)
                    if oi == 3:
                        nc.sync.dma_start(
                            outT_ap[ds(og * 512, 512), qsl].rearrange(
                                "(o p) q -> p o q", p=P
                            ),
                            obt[og],
                        )
                return go

            for og in range(2):
                for oi in range(4):
                    units.append(ou(og, oi))
            return units

        phase_a(0, 0)
        phase_a(0, 1)

        push_at = {
            (0, 0): [("a", 1)],
            (1, 0): [("a", 2)],
            (2, 0): [("a", 3), ("o", 0)],
            (3, 0): [("o", 1)],
            (3, 2): [("o", 2)],
        }

        pend_norm = None
        for qb in range(4):
            for m in range(4):
                for kind, idx in push_at.get((qb, m), []):
                    if kind == "o":
                        while any(t[0] == "n" and t[1] == idx for t, _ in filler):
                            filler.popleft()[1]()
                    if kind == "a":
                        for jp in (0, 1):
                            for u in phase_a_units(idx, jp):
                                filler.append((("a", idx), u))
                    else:
                        for u in o_units(idx):
                            filler.append((("o", idx), u))
                if m == 0:
                    flush_a(qb)
                pvt = kt_loop(qb, m)
                rc, araw = norm_a(qb, m, pvt)
                if pend_norm is not None:
                    nq, nm, nrc, nar = pend_norm
                    for u in reversed(norm_b_units(nq, nm, nrc, nar)):
                        filler.appendleft((("n", nq), u))
                pend_norm = (qb, m, rc, araw)
        while filler:
            filler.popleft()[1]()
        norm_b(*pend_norm)
        for u in o_units(3):
            u()

        if CFG.get("debug"):
            dQT, dKT, dVP, dHOPT = CFG["_dbg"]
            nc.sync.dma_start(dQT.ap(), QT if need_f16qt else QT8)
            nc.sync.dma_start(dKT.ap(), KT if need_f16qt else KT8)
            nc.sync.dma_start(dHOPT.ap(), HOPT)
            nc.sync.dma_start(dVP.ap(), VP)


# ---------------- host side ----------------

def _qk_perm():
    perm = []
    for j in range(4):
        for h4 in range(4):
            h = (j // 2) * 4 + h4
            par = j % 2
            perm.extend(h * 64 + 2 * np.arange(32) + par)
    return np.array(perm)


_PERM = _qk_perm()


def _host_tables():
    import ml_dtypes

    F16n = np.float16
    i = np.arange(32, dtype=np.float32)
    inv_freq = (THETA ** (2.0 * i / DK)).astype(np.float32)
    t = np.arange(S, dtype=np.float32)
    ang = t[None, :] / inv_freq[:, None]          # [32, S]
    sc = 1.0 / WSC if CFG["proj_hilo"] else 1.0
    cosP = np.tile(sc * np.cos(ang), (4, 1)).astype(F16n)   # [128, S]
    sinP = np.tile(sc * np.sin(ang), (4, 1)).astype(F16n)
    ident = np.eye(P, dtype=F16n)
    kk = np.arange(P)[:, None]
    qq = np.arange(512)[None, :]
    umask = (kk > qq).astype(F16n)                # [128, 512]
    negI = (NEG * np.eye(P)).astype(F16n)
    F8n = ml_dtypes.float8_e4m3fn
    negI8 = np.zeros((P, 2, P), dtype=np.float32)
    umask8 = np.zeros((P, 2, 512), dtype=np.float32)
    for ii in range(2):
        for p in range(64):
            negI8[p, ii, ii * 64 + p] = NEG8
            umask8[p, ii, :] = ((ii * 64 + p) > qq[0]).astype(np.float32)
    return cosP, sinP, ident, umask, negI, umask8.astype(F8n), negI8.astype(F8n)


def make_in_maps(x, Wq, Wk, Wv, Wo):
    import ml_dtypes

    F16n = np.float16
    F8n = ml_dtypes.float8_e4m3fn
    cosP, sinP, ident, umask, negI, umask8, negI8 = _host_tables()
    in_maps = []
    for c in range(8):
        b, hh = c // 2, c % 2
        sl = slice(hh * HD, (hh + 1) * HD)
        xT = np.ascontiguousarray(x[b].T)                      # [1024, 2048]
        wq = Wq[sl, :][_PERM]
        wk = Wk[sl, :][_PERM]
        wv = Wv[sl, :]
        wo = Wo[:, sl]                                         # [1024, 512]
        woh = np.ascontiguousarray(wo.T).reshape(4, P, D)
        m = {
            "xh": xT.reshape(8, P, S).astype(F16n),
            "wqh": np.ascontiguousarray(wq.T).reshape(8, P, HD).astype(F16n),
            "wkh": np.ascontiguousarray(wk.T).reshape(8, P, HD).astype(F16n),
            "wvh": np.ascontiguousarray(wv.T).reshape(8, P, HD).astype(F16n),
            "woh": woh.astype(F16n),
            "cosP": cosP, "sinP": sinP, "ident": ident,
            "umask": umask, "negI": negI, "umask8": umask8, "negI8": negI8,
        }
        if CFG["proj_hilo"]:
            def drfmt(a, n):
                return np.ascontiguousarray(a).reshape(4, 2, P, n).transpose(2, 0, 1, 3)

            def hilo8(a, n, scale):
                a = drfmt(a, n) * scale
                hi = a.astype(F8n)
                lo = (a - hi.astype(np.float32)).astype(F8n)
                return hi, lo

            m["x8h"], m["x8l"] = hilo8(xT, S, 1.0)
            m["wq8h"], m["wq8l"] = hilo8(wq.T, HD, WSC)
            m["wk8h"], m["wk8l"] = hilo8(wk.T, HD, WSC)
            m["wv8h"], m["wv8l"] = hilo8(wv.T, HD, WSC)
        in_maps.append(m)
    return in_maps


def gather_out(core_outs):
    out = np.empty((B, S, D), dtype=np.float32)
    for b in range(B):
        out[b] = (core_outs[2 * b]["outT"] + core_outs[2 * b + 1]["outT"]).T
    return out


_NC_CACHE = {}


def kernel(x, Wq, Wk, Wv, Wo):
    x = np.asarray(x, dtype=np.float32)
    Wq = np.asarray(Wq, dtype=np.float32)
    Wk = np.asarray(Wk, dtype=np.float32)
    Wv = np.asarray(Wv, dtype=np.float32)
    Wo = np.asarray(Wo, dtype=np.float32)
    if "nc" not in _NC_CACHE:
        _NC_CACHE["nc"] = build_attention_nc()
    nc = _NC_CACHE["nc"]
    in_maps = make_in_maps(x, Wq, Wk, Wv, Wo)
    res = run_bass_kernel_spmd(nc, in_maps, core_ids=list(range(8)))
    return gather_out(res.results)


# revision 5
# speedup vs baseline: 1.1481x; 1.0089x over previous
"""Causal MHA (B=4, S=2048, D=1024, H=16, RoPE) on 8 trn2 cores — v2.

Sharding: core c -> batch c//2, head-half c%2 (8 heads / 512 dims per core).

Design vs v1 baseline:
  - Q/K weights host-permuted into even/odd 32-blocks per 4-head group so
    RoPE needs NO partition-swap DMA: psum tile pairs (j, j+1) hold the
    even/odd halves at identical partition indices and the rotation is plain
    elementwise tensor ops (fp16, 2x DVE rate).
  - fp16 downstream (P, V, attn, Wo): 1 cyc/row matmuls.
  - Optional fp8e4m3 DoubleRow scores (0.5 cyc/row): the even/odd layout is
    natively DR-compatible ([32 parts, 2 j-slots, t] APs), no re-layout DMA.
  - PV in [q, dv] orientation (moving dim 65): psum accumulators
    [128, 2qt, 130] with ones-column densities; normalization via
    per-partition recips + diag-matmul transpose back to [d', t].
  - Software-pipelined emission: normalize(m) deferred into (m+1)'s kt loop,
    O-proj(qb) into qb+1, phase-A chunks interleaved at m boundaries, so the
    in-order PE queue never head-of-line blocks on DVE/Act chains.
"""

import numpy as np

import concourse.bass as bass
import concourse.bacc as bacc
import concourse.mybir as mybir
import concourse.tile as tile
from concourse.bass import ds, ts
from concourse.bass_utils import run_bass_kernel_spmd

F32 = mybir.dt.float32
F16 = mybir.dt.float16
F8 = mybir.dt.float8e4
DR = mybir.MatmulPerfMode.DoubleRow

B, S, D, H, DK = 4, 2048, 1024, 16, 64
THETA = 10000.0
NH = 8
HD = NH * DK
P = 128
NEG = -28000.0
NEG8 = -240.0

CFG = {
    "s_dr": "ge512",   # "none" | "ge512" | "all"
    "proj_dr": False,
    "proj_hilo": True,  # hi+lo fp8e4m3 DoubleRow projections (W pre-scaled x16)
    "rope_mul_pool": False,
}
WSC = 16.0  # weight pre-scale for hi/lo fp8 (folded out via rope tables / araw)


def build_attention_nc(nrep=1):
    nc = bacc.Bacc("TRN2", target_bir_lowering=False, debug=False)

    xh = nc.dram_tensor("xh", [D // P, P, S], F16, kind="ExternalInput")
    wqh = nc.dram_tensor("wqh", [D // P, P, HD], F16, kind="ExternalInput")
    wkh = nc.dram_tensor("wkh", [D // P, P, HD], F16, kind="ExternalInput")
    wvh = nc.dram_tensor("wvh", [D // P, P, HD], F16, kind="ExternalInput")
    woh = nc.dram_tensor("woh", [HD // P, P, D], F16, kind="ExternalInput")
    cosP = nc.dram_tensor("cosP", [P, S], F16, kind="ExternalInput")
    sinP = nc.dram_tensor("sinP", [P, S], F16, kind="ExternalInput")
    ident = nc.dram_tensor("ident", [P, P], F16, kind="ExternalInput")
    umask = nc.dram_tensor("umask", [P, 512], F16, kind="ExternalInput")
    negI = nc.dram_tensor("negI", [P, P], F16, kind="ExternalInput")
    umask8 = nc.dram_tensor("umask8", [P, 2, 512], F8, kind="ExternalInput")
    negI8 = nc.dram_tensor("negI8", [P, 2, P], F8, kind="ExternalInput")
    outT = nc.dram_tensor("outT", [D, S], F32, kind="ExternalOutput")
    if CFG.get("debug"):
        dQT = nc.dram_tensor("dQT", [P, 4, S], F16, kind="ExternalOutput")
        dKT = nc.dram_tensor("dKT", [P, 4, S], F16, kind="ExternalOutput")
        dVP = nc.dram_tensor("dVP", [P, 16, NH, DK + 1], F16, kind="ExternalOutput")
        dHOPT = nc.dram_tensor("dHOPT", [P, 4, S], F16, kind="ExternalOutput")
        CFG["_dbg"] = (dQT, dKT, dVP, dHOPT)

    hl = None
    if CFG["proj_hilo"]:
        hl = {}
        for nm in ("x8h", "x8l"):
            hl[nm] = nc.dram_tensor(nm, [P, D // 256, 2, S], F8, kind="ExternalInput")
        for nm in ("wq8h", "wq8l", "wk8h", "wk8l", "wv8h", "wv8l"):
            hl[nm] = nc.dram_tensor(nm, [P, D // 256, 2, HD], F8, kind="ExternalInput")

    with tile.TileContext(nc) as tc:
        if nrep == 1:
            _attn_tile(tc, xh, hl, wqh, wkh, wvh, woh, cosP, sinP,
                       ident, umask, negI, umask8, negI8, outT)
        else:
            with tc.For_i(0, nrep, 1):
                _attn_tile(tc, xh, hl, wqh, wkh, wvh, woh, cosP,
                           sinP, ident, umask, negI, umask8, negI8, outT)
    nc.compile()
    return nc


def _attn_tile(tc, xh, hl, wqh, wkh, wvh, woh, cosP, sinP, ident,
               umask, negI, umask8, negI8, outT):
    nc = tc.nc
    s_dr = CFG["s_dr"]
    hilo = CFG["proj_hilo"]
    need_f8 = s_dr != "none"
    need_f16qt = s_dr != "all"

    with (
        tc.tile_pool(name="per", bufs=1) as per,
        tc.tile_pool(name="ab", bufs=8) as abp,
        tc.tile_pool(name="rt", bufs=6) as rtp,
        tc.tile_pool(name="ptp", bufs=4) as ptp,
        tc.tile_pool(name="arp", bufs=4) as arp,
        tc.tile_pool(name="rcp", bufs=2) as rcp,
        tc.tile_pool(name="dgp", bufs=6) as dgp,
        tc.tile_pool(name="obp", bufs=3) as obp,
        tc.tile_pool(name="psS", bufs=2, space="PSUM") as psS,
        tc.tile_pool(name="psPV", bufs=2, space="PSUM") as psPV,
        tc.tile_pool(name="psC", bufs=2, space="PSUM") as psC,
    ):
        # ---------------- persistent tiles + input DMAs ----------------
        if need_f16qt:
            QT = per.tile([P, 4, S], F16, tag="QT")
            KT = per.tile([P, 4, S], F16, tag="KT")
        else:
            QT = KT = None
        if need_f8:
            QT8 = per.tile([P, 4, S], F8, tag="QT8")
            KT8 = per.tile([P, 4, S], F8, tag="KT8")
        else:
            QT8 = KT8 = None
        VP = per.tile([P, 16, NH, DK + 1], F16, tag="VP")
        HOPT = per.tile([P, 4, S], F16, tag="HOPT")

        # ones column of VP (col 0 per head)
        nc.vector.memset(VP[:, :, :, 0:1], 1.0)

        # DMAs ordered so phase_a(0, 0) can start as early as possible:
        # j-pair-0 halves of wq first, then the first x chunk, wk, cos/sin, wv.
        if hilo:
            hsb = {}
            for nm in ("x8h", "x8l"):
                hsb[nm] = per.tile([P, 4, 2, S], F8, tag=nm, name=nm)
            for nm in ("wq8h", "wq8l", "wk8h", "wk8l", "wv8h", "wv8l"):
                hsb[nm] = per.tile([P, 4, 2, HD], F8, tag=nm, name=nm)

            def hsrc(nm):
                return hl[nm].ap()

            for nm in ("wq8h", "wq8l"):
                nc.sync.dma_start(hsb[nm][:, :, :, 0:256], hsrc(nm)[:, :, :, 0:256])
            for nm in ("x8h", "x8l"):
                nc.sync.dma_start(hsb[nm][:, :, :, 0:512], hsrc(nm)[:, :, :, 0:512])
            for nm in ("wk8h", "wk8l"):
                nc.sync.dma_start(hsb[nm][:, :, :, 0:256], hsrc(nm)[:, :, :, 0:256])
            for nm in ("wv8h", "wv8l"):
                nc.sync.dma_start(hsb[nm][:, :, 0:1, :], hsrc(nm)[:, :, 0:1, :])
            cos_sb = per.tile([P, S], F16, tag="cos")
            sin_sb = per.tile([P, S], F16, tag="sin")
            nc.sync.dma_start(cos_sb, cosP.ap())
            nc.sync.dma_start(sin_sb, sinP.ap())
            for nm in ("wv8h", "wv8l"):
                nc.sync.dma_start(hsb[nm][:, :, 1:2, :], hsrc(nm)[:, :, 1:2, :])
            for nm in ("wq8h", "wq8l", "wk8h", "wk8l"):
                nc.sync.dma_start(hsb[nm][:, :, :, 256:512], hsrc(nm)[:, :, :, 256:512])
            for tci in range(1, 4):
                tsl = ds(tci * 512, 512)
                for nm in ("x8h", "x8l"):
                    nc.sync.dma_start(hsb[nm][:, :, :, tsl], hsrc(nm)[:, :, :, tsl])
        else:
            xh_sb = per.tile([P, 8, S], F16, tag="xh")
            xh_src = xh.ap().rearrange("o p t -> p o t")
            wq_sb = per.tile([P, 8, HD], F16, tag="wq")
            wk_sb = per.tile([P, 8, HD], F16, tag="wk")
            wq_src = wqh.ap().rearrange("o p d -> p o d")
            wk_src = wkh.ap().rearrange("o p d -> p o d")
            for jp in (0, 1):
                dsl = ds(jp * 256, 256)
                nc.sync.dma_start(wq_sb[:, :, dsl], wq_src[:, :, dsl])
                nc.sync.dma_start(wk_sb[:, :, dsl], wk_src[:, :, dsl])
            nc.sync.dma_start(xh_sb[:, :, 0:512], xh_src[:, :, 0:512])
            cos_sb = per.tile([P, S], F16, tag="cos")
            sin_sb = per.tile([P, S], F16, tag="sin")
            nc.sync.dma_start(cos_sb, cosP.ap())
            nc.sync.dma_start(sin_sb, sinP.ap())
            wv_sb = per.tile([P, 8, HD], F16, tag="wv")
            nc.sync.dma_start(wv_sb, wvh.ap().rearrange("o p d -> p o d"))
            for tci in range(1, 4):
                tsl = ds(tci * 512, 512)
                nc.sync.dma_start(xh_sb[:, :, tsl], xh_src[:, :, tsl])
        wo_sb = per.tile([P, 4, D], F16, tag="wo")
        nc.sync.dma_start(wo_sb, woh.ap().rearrange("m p o -> p m o"))
        I_sb = per.tile([P, P], F16, tag="I")
        nc.sync.dma_start(I_sb, ident.ap())
        z_sb = per.tile([P, 512], F16, tag="z")
        nc.vector.memset(z_sb[0:1, :], 0.0)
        um_sb = per.tile([P, 512], F16, tag="um")
        nc.sync.dma_start(um_sb, umask.ap())
        nI_sb = per.tile([P, P], F16, tag="nI")
        nc.sync.dma_start(nI_sb, negI.ap())
        if need_f8:
            um8_sb = per.tile([P, 2, 512], F8, tag="um8")
            nc.sync.dma_start(um8_sb, umask8.ap())
            nI8_sb = per.tile([P, 2, P], F8, tag="nI8")
            nc.sync.dma_start(nI8_sb, negI8.ap())

        outT_ap = outT.ap()

        def act_copy(out, in_):
            nc.scalar.activation(out, in_, mybir.ActivationFunctionType.Copy)

        # ---------------- phase A half-chunk (t block 512, one j-pair) ----
        def phase_a_units(tci, jp):
            """Yield small closures: 2 Q-proj tiles, ropeQ, 2 K-proj tiles,
            ropeK, 2 V tiles. Emitted piecemeal between attention kts."""
            tsl = ds(tci * 512, 512)
            abt = {}

            def proj_unit(wname, j):
              def go():
                    ps = psC.tile([P, 512], F32, tag="c", name=f"pa{tci}{j}")
                    if hilo:
                        first = True
                        for c in range(4):
                            for wsfx, xsfx in (("h", "h"), ("h", "l"), ("l", "h")):
                                nc.tensor.matmul(
                                    ps,
                                    lhsT=hsb[f"{wname}8{wsfx}"][:, c, :, ts(j, P)],
                                    rhs=hsb[f"x8{xsfx}"][:, c, :, tsl],
                                    start=first, stop=(c == 3 and wsfx == "l"),
                                    perf_mode=DR,
                                )
                                first = False
                    else:
                        w = wq_sb if wname == "wq" else wk_sb
                        for o in range(8):
                            nc.tensor.matmul(
                                ps,
                                lhsT=w[:, o, ts(j, P)],
                                rhs=xh_sb[:, o, tsl],
                                start=(o == 0), stop=(o == 7),
                            )
                    ab = abp.tile([P, 512], F16, tag="ab", name=f"ab{j}")
                    nc.vector.tensor_copy(ab, ps)
                    abt[j] = ab
              return go

            def v_unit(tt):
              def go():
                    psv = psC.tile([P, 512], F32, tag="c", name=f"pvv{tci}{tt}")
                    xsl = ds(tci * 512 + tt * P, P)
                    if hilo:
                        first = True
                        for c in range(4):
                            for xsfx, wsfx in (("h", "h"), ("h", "l"), ("l", "h")):
                                nc.tensor.matmul(
                                    psv,
                                    lhsT=hsb[f"x8{xsfx}"][:, c, :, xsl],
                                    rhs=hsb[f"wv8{wsfx}"][:, c, :, :],
                                    start=first, stop=(c == 3 and xsfx == "l"),
                                    perf_mode=DR,
                                )
                                first = False
                    else:
                        for o in range(8):
                            nc.tensor.matmul(
                                psv,
                                lhsT=xh_sb[:, o, xsl],
                                rhs=wv_sb[:, o, :],
                                start=(o == 0), stop=(o == 7),
                            )
                    nc.vector.tensor_copy(
                        VP[:, tci * 4 + tt, :, 1:],
                        psv.rearrange("p (h c) -> p h c", c=DK),
                    )
              return go

            def rope_unit(tensor):
              def go():
                    mul_eng = nc.gpsimd if CFG["rope_mul_pool"] else nc.vector
                    je, jo = 2 * jp, 2 * jp + 1
                    A, Bb = abt[je] if tensor == 0 else abt[je + 10], None
                    # resolved below
              return go

            yield proj_unit("wq", 2 * jp)
            yield proj_unit("wq", 2 * jp + 1)
            yield _rope_closure(tci, jp, abt, 0)
            yield proj_unit("wk", 2 * jp)
            yield proj_unit("wk", 2 * jp + 1)
            yield _rope_closure(tci, jp, abt, 1)
            yield v_unit(2 * jp)
            yield v_unit(2 * jp + 1)

        def _unused(tci, jp):
            tsl = ds(tci * 512, 512)
            abt = []
            for wname in ("wq", "wk"):
                for j in (2 * jp, 2 * jp + 1):
                    ps = psC.tile([P, 512], F32, tag="c", name=f"pa{tci}{j}")
                    if hilo:
                        first = True
                        for c in range(4):
                            for wsfx, xsfx in (("h", "h"), ("h", "l"), ("l", "h")):
                                nc.tensor.matmul(
                                    ps,
                                    lhsT=hsb[f"{wname}8{wsfx}"][:, c, :, ts(j, P)],
                                    rhs=hsb[f"x8{xsfx}"][:, c, :, tsl],
                                    start=first, stop=(c == 3 and wsfx == "l"),
                                    perf_mode=DR,
                                )
                                first = False
                    else:
                        w = wq_sb if wname == "wq" else wk_sb
                        for o in range(8):
                            nc.tensor.matmul(
                                ps,
                                lhsT=w[:, o, ts(j, P)],
                                rhs=xh_sb[:, o, tsl],
                                start=(o == 0), stop=(o == 7),
                            )
                    ab = abp.tile([P, 512], F16, tag="ab", name=f"ab{j}")
                    act_copy(ab, ps)
                    abt.append(ab)
            # V projection for two t-subtiles
            for tt in (2 * jp, 2 * jp + 1):
                psv = psC.tile([P, 512], F32, tag="c", name=f"pvv{tci}{tt}")
                xsl = ds(tci * 512 + tt * P, P)
                if hilo:
                    first = True
                    for c in range(4):
                        for xsfx, wsfx in (("h", "h"), ("h", "l"), ("l", "h")):
                            nc.tensor.matmul(
                                psv,
                                lhsT=hsb[f"x8{xsfx}"][:, c, :, xsl],
                                rhs=hsb[f"wv8{wsfx}"][:, c, :, :],
                                start=first, stop=(c == 3 and xsfx == "l"),
                                perf_mode=DR,
                            )
                            first = False
                else:
                    for o in range(8):
                        nc.tensor.matmul(
                            psv,
                            lhsT=xh_sb[:, o, xsl],
                            rhs=wv_sb[:, o, :],
                            start=(o == 0), stop=(o == 7),
                        )
                act_copy(
                    VP[:, tci * 4 + tt, :, 1:],
                    psv.rearrange("p (h c) -> p h c", c=DK),
                )
            # RoPE for this j-pair, both tensors
            mul_eng = nc.gpsimd if CFG["rope_mul_pool"] else nc.vector
            for tensor in (0, 1):
                A, Bb = abt[2 * tensor], abt[2 * tensor + 1]
                dst = (QT, KT)[tensor]
                dst8 = (QT8, KT8)[tensor]
                je, jo = 2 * jp, 2 * jp + 1
                t1 = rtp.tile([P, 512], F16, tag="rt", name="t1")
                t2 = rtp.tile([P, 512], F16, tag="rt", name="t2")
                mul_eng.tensor_mul(t1, cos_sb[:, tsl], A)
                mul_eng.tensor_mul(t2, sin_sb[:, tsl], Bb)
                t3 = rtp.tile([P, 512], F16, tag="rt", name="t3")
                t4 = rtp.tile([P, 512], F16, tag="rt", name="t4")
                mul_eng.tensor_mul(t3, sin_sb[:, tsl], A)
                mul_eng.tensor_mul(t4, cos_sb[:, tsl], Bb)
                if need_f16qt:
                    nc.vector.tensor_sub(dst[:, je, tsl], t1, t2)
                    nc.vector.tensor_add(dst[:, jo, tsl], t3, t4)
                    if need_f8:
                        nc.gpsimd.tensor_copy(dst8[:, je, tsl], dst[:, je, tsl])
                        nc.gpsimd.tensor_copy(dst8[:, jo, tsl], dst[:, jo, tsl])
                else:
                    nc.vector.tensor_sub(dst8[:, je, tsl], t1, t2)
                    nc.vector.tensor_add(dst8[:, jo, tsl], t3, t4)

        # ---------------- attention stages ----------------
        def kt_loop(qb, m):
            """S + exp + PV accumulation for head pair m of q block qb.
            Returns state consumed by normalize()."""
            use_dr = s_dr == "all" or (s_dr == "ge512" and qb >= 1)
            pvt = [
                psPV.tile([P, 2, 130], F32, tag="pv", name=f"pv{qb}{m}{i}")
                for i in range(2)
            ]
            # start=True zeroes the whole 2KB bank (ZERO_REGION_SIZE), so a
            # shared-bank tile is zeroed once up front and all sub-region
            # accumulations use start=False.
            for i in range(2):
                nc.tensor.matmul(
                    pvt[i], lhsT=z_sb[0:1, 0:P], rhs=z_sb[0:1, 0:260],
                    start=True, stop=False, skip_group_check=True,
                )
            nkt = 4 * qb + 4
            pend_pv = None  # (kt, pt2): PV deferred one kt so S(kt+1)
                            # precedes PV(kt) in the in-order PE queue

            def emit_pv(kt, pt2):
                roff_ = kt - 4 * qb
                for e in (0, 1):
                    h = 2 * m + e
                    for qt in range(4):
                        if qt < roff_:
                            continue
                        nc.tensor.matmul(
                            pvt[qt // 2][:, qt % 2, ds(e * 65, 65)],
                            lhsT=pt2[:, e, ds(qt * P, P)],
                            rhs=VP[:, kt, h, :],
                            start=False,
                            stop=(kt == 4 * qb + qt),
                            skip_group_check=True,
                        )

            for kt in range(nkt):
                roff = kt - 4 * qb
                c0 = P * max(roff, 0)
                live = 512 - c0
                s2 = psS.tile([P, 2, 512], F32, tag="s", name=f"s{qb}{m}{kt}")
                for e in (0, 1):
                    h = 2 * m + e
                    g, h4 = h // 4, h % 4
                    pr = ds(h4 * 32, 32)
                    tpos = (h4 * 32, 0)
                    if use_dr:
                        nc.tensor.matmul(
                            s2[:, e, c0:],
                            lhsT=KT8[pr, ds(2 * g, 2), ts(kt, P)],
                            rhs=QT8[pr, ds(2 * g, 2), ds(qb * 512 + c0, live)],
                            start=True, stop=(roff < 0),
                            perf_mode=DR,
                            skip_group_check=True,
                            tile_position=tpos,
                        )
                    else:
                        nc.tensor.matmul(
                            s2[:, e, c0:],
                            lhsT=KT[pr, 2 * g, ts(kt, P)],
                            rhs=QT[pr, 2 * g, ds(qb * 512 + c0, live)],
                            start=True, stop=False,
                            skip_group_check=True,
                            tile_position=tpos,
                        )
                        nc.tensor.matmul(
                            s2[:, e, c0:],
                            lhsT=KT[pr, 2 * g + 1, ts(kt, P)],
                            rhs=QT[pr, 2 * g + 1, ds(qb * 512 + c0, live)],
                            start=False, stop=(roff < 0),
                            skip_group_check=True,
                            tile_position=tpos,
                        )
                    if roff >= 0:
                        if use_dr:
                            nc.tensor.matmul(
                                s2[:, e, ds(c0, P)],
                                lhsT=nI8_sb,
                                rhs=um8_sb[:, :, 0:P],
                                start=False, stop=True,
                                perf_mode=DR,
                                skip_group_check=True,
                            )
                        else:
                            nc.tensor.matmul(
                                s2[:, e, ds(c0, P)],
                                lhsT=nI_sb,
                                rhs=um_sb[:, 0:P],
                                start=False, stop=True,
                                skip_group_check=True,
                            )
                pt2 = ptp.tile([P, 2, 512], F16, tag="pt")
                nc.scalar.activation(
                    pt2[:, :, c0:], s2[:, :, c0:],
                    mybir.ActivationFunctionType.Exp, scale=0.125,
                )
                if pend_pv is not None:
                    emit_pv(*pend_pv)
                pend_pv = (kt, pt2)
                drain(1)
            emit_pv(*pend_pv)
            return pvt

        def norm_a(qb, m, pvt):
            """Drain pv psum right away: recip denominators to SBUF + raw
            fp16 copy. DVE-only, so the PE queue is not blocked; frees the
            pv psum ring for the next kt_loop."""
            rc = rcp.tile([P, 2, 2, 2], F32, tag="rc", name=f"rc{qb}{m}")
            araw = []
            for i in (0, 1):
                rsl = pvt[i].rearrange("p a (b c) -> p a b c", c=65)[:, :, :, 0]
                nc.vector.reciprocal(rc[:, i], rsl)
                ar = arp.tile([P, 2, 130], F16, tag="ar", name=f"ar{i}")
                if hilo:
                    nc.vector.tensor_scalar_mul(ar, pvt[i], 1.0 / WSC)
                else:
                    nc.vector.tensor_copy(ar, pvt[i])
                araw.append(ar)
            return rc, araw

        def norm_b_units(qb, m, rc, araw):
            """Deferred: diag builds + scaled transpose into HOPT, split into
            two filler units so the PE queue never blocks long on them."""
            qsl = ds(qb * 512, 512)
            st = {}

            def half(which):
                def go():
                    if which == 0:
                        st["tp"] = psC.tile([P, 512], F32, tag="c",
                                            name=f"tp{qb}{m}")
                        nc.tensor.matmul(
                            st["tp"], lhsT=z_sb[0:1, 0:P], rhs=z_sb[0:1, :],
                            start=True, stop=False, skip_group_check=True,
                        )
                    tp = st["tp"]
                    for qt in (0, 1) if which == 0 else (2, 3):
                        for e in (0, 1):
                            dg = dgp.tile([P, P], F16, tag="dg")
                            nc.vector.tensor_scalar_mul(
                                dg, I_sb, rc[:, qt // 2, qt % 2, ds(e, 1)]
                            )
                            nc.tensor.matmul(
                                tp[ds(e * DK, DK), ts(qt, P)],
                                lhsT=araw[qt // 2][:, qt % 2, ds(e * 65 + 1, DK)],
                                rhs=dg,
                                start=False, stop=(qt == 3),
                                skip_group_check=True,
                            )
                    if which == 1:
                        nc.vector.tensor_copy(HOPT[:, m, qsl], tp)
                return go

            return [half(0), half(1)]

        def norm_b(qb, m, rc, araw):
            for u in norm_b_units(qb, m, rc, araw):
                u()

        def o_proj(qb):
            qsl = ds(qb * 512, 512)
            for og in range(2):
                ob = obp.tile([P, 4, 512], F32, tag="ob")
                for oi in range(4):
                    ot = og * 4 + oi
                    psc = psC.tile([P, 512], F32, tag="c", name=f"oc{qb}{ot}")
                    for mm in range(4):
                        nc.tensor.matmul(
                            psc,
                            lhsT=wo_sb[:, mm, ts(ot, P)],
                            rhs=HOPT[:, mm, qsl],
                            start=(mm == 0), stop=(mm == 3),
                        )
                    nc.vector.tensor_copy(ob[:, oi, :], psc)
                nc.sync.dma_start(
                    outT_ap[ds(og * 512, 512), qsl].rearrange(
                        "(o p) q -> p o q", p=P
                    ),
                    ob,
                )

        # ---------------- emission schedule (software-pipelined) ----------
        # Filler units (phase-A pieces, o_proj pieces) are drained one per kt
        # inside the attention loops so the PE always interleaves projection
        # work with the exp-feeding S-matmuls instead of blocking Act behind
        # multi-microsecond projection bursts.
        import os
        from collections import deque

        filler = deque()  # (tag, closure); tag = ("a", tci) or ("o", qb)

        def drain(n=1):
            for _ in range(n):
                if filler:
                    filler.popleft()[1]()

        def flush_a(tci):
            while any(t[0] == "a" and t[1] <= tci for t, _ in filler):
                filler.popleft()[1]()

        def o_units(qb):
            qsl = ds(qb * 512, 512)
            units = []
            obt = {}

            def ou(og, oi):
                def go():
                    if oi == 0:
                        obt[og] = obp.tile([P, 4, 512], F32, tag="ob",
                                           name=f"ob{qb}{og}")
                    ot = og * 4 + oi
                    psc = psC.tile([P, 512], F32, tag="c", name=f"oc{qb}{ot}")
                    for mm in range(4):
                        nc.tensor.matmul(
                            psc,
                            lhsT=wo_sb[:, mm, ts(ot, P)],
                            rhs=HOPT[:, mm, qsl],
                            start=(mm == 0), stop=(mm == 3),
                        )
                    nc.vector.tensor_copy(obt[og][:, oi, :], psc)
                    if oi == 3:
                        nc.sync.dma_start(
                            outT_ap[ds(og * 512, 512), qsl].rearrange(
                                "(o p) q -> p o q", p=P
                            ),
                            obt[og],
                        )
                return go

            for og in range(2):
                for oi in range(4):
                    units.append(ou(og, oi))
            return units

        phase_a(0, 0)
        phase_a(0, 1)

        push_at = {
            (0, 0): [("a", 1)],
            (1, 0): [("a", 2)],
            (2, 0): [("a", 3), ("o", 0)],
            (3, 0): [("o", 1)],
            (3, 2): [("o", 2)],
        }

        pend_norm = None
        for qb in range(4):
            for m in range(4):
                for kind, idx in push_at.get((qb, m), []):
                    if kind == "o":
                        while any(t[0] == "n" and t[1] == idx for t, _ in filler):
                            filler.popleft()[1]()
                    if kind == "a":
                        for jp in (0, 1):
                            for u in phase_a_units(idx, jp):
                                filler.append((("a", idx), u))
                    else:
                        for u in o_units(idx):
                            filler.append((("o", idx), u))
                if m == 0:
                    flush_a(qb)
                pvt = kt_loop(qb, m)
                rc, araw = norm_a(qb, m, pvt)
                if pend_norm is not None:
                    nq, nm, nrc, nar = pend_norm
                    for u in reversed(norm_b_units(nq, nm, nrc, nar)):
                        filler.appendleft((("n", nq), u))
                pend_norm = (qb, m, rc, araw)
        while filler:
            filler.popleft()[1]()
        norm_b(*pend_norm)
        for u in o_units(3):
            u()

        if CFG.get("debug"):
            dQT, dKT, dVP, dHOPT = CFG["_dbg"]
            nc.sync.dma_start(dQT.ap(), QT if need_f16qt else QT8)
            nc.sync.dma_start(dKT.ap(), KT if need_f16qt else KT8)
            nc.sync.dma_start(dHOPT.ap(), HOPT)
            nc.sync.dma_start(dVP.ap(), VP)


# ---------------- host side ----------------

def _qk_perm():
    perm = []
    for j in range(4):
        for h4 in range(4):
            h = (j // 2) * 4 + h4
            par = j % 2
            perm.extend(h * 64 + 2 * np.arange(32) + par)
    return np.array(perm)


_PERM = _qk_perm()


def _host_tables():
    import ml_dtypes

    F16n = np.float16
    i = np.arange(32, dtype=np.float32)
    inv_freq = (THETA ** (2.0 * i / DK)).astype(np.float32)
    t = np.arange(S, dtype=np.float32)
    ang = t[None, :] / inv_freq[:, None]          # [32, S]
    sc = 1.0 / WSC if CFG["proj_hilo"] else 1.0
    cosP = np.tile(sc * np.cos(ang), (4, 1)).astype(F16n)   # [128, S]
    sinP = np.tile(sc * np.sin(ang), (4, 1)).astype(F16n)
    ident = np.eye(P, dtype=F16n)
    kk = np.arange(P)[:, None]
    qq = np.arange(512)[None, :]
    umask = (kk > qq).astype(F16n)                # [128, 512]
    negI = (NEG * np.eye(P)).astype(F16n)
    F8n = ml_dtypes.float8_e4m3fn
    negI8 = np.zeros((P, 2, P), dtype=np.float32)
    umask8 = np.zeros((P, 2, 512), dtype=np.float32)
    for ii in range(2):
        for p in range(64):
            negI8[p, ii, ii * 64 + p] = NEG8
            umask8[p, ii, :] = ((ii * 64 + p) > qq[0]).astype(np.float32)
    return cosP, sinP, ident, umask, negI, umask8.astype(F8n), negI8.astype(F8n)


def make_in_maps(x, Wq, Wk, Wv, Wo):
    import ml_dtypes

    F16n = np.float16
    F8n = ml_dtypes.float8_e4m3fn
    cosP, sinP, ident, umask, negI, umask8, negI8 = _host_tables()
    in_maps = []
    for c in range(8):
        b, hh = c // 2, c % 2
        sl = slice(hh * HD, (hh + 1) * HD)
        xT = np.ascontiguousarray(x[b].T)                      # [1024, 2048]
        wq = Wq[sl, :][_PERM]
        wk = Wk[sl, :][_PERM]
        wv = Wv[sl, :]
        wo = Wo[:, sl]                                         # [1024, 512]
        woh = np.ascontiguousarray(wo.T).reshape(4, P, D)
        m = {
            "xh": xT.reshape(8, P, S).astype(F16n),
            "wqh": np.ascontiguousarray(wq.T).reshape(8, P, HD).astype(F16n),
            "wkh": np.ascontiguousarray(wk.T).reshape(8, P, HD).astype(F16n),
            "wvh": np.ascontiguousarray(wv.T).reshape(8, P, HD).astype(F16n),
            "woh": woh.astype(F16n),
            "cosP": cosP, "sinP": sinP, "ident": ident,
            "umask": umask, "negI": negI, "umask8": umask8, "negI8": negI8,
        }
        if CFG["proj_hilo"]:
            def drfmt(a, n):
                return np.ascontiguousarray(a).reshape(4, 2, P, n).transpose(2, 0, 1, 3)

            def hilo8(a, n, scale):
                a = drfmt(a, n) * scale
                hi = a.astype(F8n)
                lo = (a - hi.astype(np.float32)).astype(F8n)
                return hi, lo

            m["x8h"], m["x8l"] = hilo8(xT, S, 1.0)
            m["wq8h"], m["wq8l"] = hilo8(wq.T, HD, WSC)
            m["wk8h"], m["wk8l"] = hilo8(wk.T, HD, WSC)
            m["wv8h"], m["wv8l"] = hilo8(wv.T, HD, WSC)
        in_maps.append(m)
    return in_maps


def gather_out(core_outs):
    out = np.empty((B, S, D), dtype=np.float32)
    for b in range(B):
        out[b] = (core_outs[2 * b]["outT"] + core_outs[2 * b + 1]["outT"]).T
    return out


_NC_CACHE = {}


def kernel(x, Wq, Wk, Wv, Wo):
    x = np.asarray(x, dtype=np.float32)
    Wq = np.asarray(Wq, dtype=np.float32)
    Wk = np.asarray(Wk, dtype=np.float32)
    Wv = np.asarray(Wv, dtype=np.float32)
    Wo = np.asarray(Wo, dtype=np.float32)
    if "nc" not in _NC_CACHE:
        _NC_CACHE["nc"] = build_attention_nc()
    nc = _NC_CACHE["nc"]
    in_maps = make_in_maps(x, Wq, Wk, Wv, Wo)
    res = run_bass_kernel_spmd(nc, in_maps, core_ids=list(range(8)))
    return gather_out(res.results)


# revision 6
# speedup vs baseline: 1.1672x; 1.0167x over previous
"""Causal MHA (B=4, S=2048, D=1024, H=16, RoPE) on 8 trn2 cores — v2.

Sharding: core c -> batch c//2, head-half c%2 (8 heads / 512 dims per core).

Design vs v1 baseline:
  - Q/K weights host-permuted into even/odd 32-blocks per 4-head group so
    RoPE needs NO partition-swap DMA: psum tile pairs (j, j+1) hold the
    even/odd halves at identical partition indices and the rotation is plain
    elementwise tensor ops (fp16, 2x DVE rate).
  - fp16 downstream (P, V, attn, Wo): 1 cyc/row matmuls.
  - Optional fp8e4m3 DoubleRow scores (0.5 cyc/row): the even/odd layout is
    natively DR-compatible ([32 parts, 2 j-slots, t] APs), no re-layout DMA.
  - PV in [q, dv] orientation (moving dim 65): psum accumulators
    [128, 2qt, 130] with ones-column densities; normalization via
    per-partition recips + diag-matmul transpose back to [d', t].
  - Software-pipelined emission: normalize(m) deferred into (m+1)'s kt loop,
    O-proj(qb) into qb+1, phase-A chunks interleaved at m boundaries, so the
    in-order PE queue never head-of-line blocks on DVE/Act chains.
"""

import numpy as np

import concourse.bass as bass
import concourse.bacc as bacc
import concourse.mybir as mybir
import concourse.tile as tile
from concourse.bass import ds, ts
from concourse.bass_utils import run_bass_kernel_spmd

F32 = mybir.dt.float32
F16 = mybir.dt.float16
F8 = mybir.dt.float8e4
DR = mybir.MatmulPerfMode.DoubleRow

B, S, D, H, DK = 4, 2048, 1024, 16, 64
THETA = 10000.0
NH = 8
HD = NH * DK
P = 128
NEG = -28000.0
NEG8 = -240.0

CFG = {
    "s_dr": "ge512",   # "none" | "ge512" | "all"
    "proj_dr": False,
    "proj_hilo": True,  # hi+lo fp8e4m3 DoubleRow projections (W pre-scaled x16)
    "rope_mul_pool": False,
}
WSC = 16.0  # weight pre-scale for hi/lo fp8 (folded out via rope tables / araw)


def build_attention_nc(nrep=1):
    nc = bacc.Bacc("TRN2", target_bir_lowering=False, debug=False)

    xh = nc.dram_tensor("xh", [D // P, P, S], F16, kind="ExternalInput")
    wqh = nc.dram_tensor("wqh", [D // P, P, HD], F16, kind="ExternalInput")
    wkh = nc.dram_tensor("wkh", [D // P, P, HD], F16, kind="ExternalInput")
    wvh = nc.dram_tensor("wvh", [D // P, P, HD], F16, kind="ExternalInput")
    woh = nc.dram_tensor("woh", [HD // P, P, D], F16, kind="ExternalInput")
    cosP = nc.dram_tensor("cosP", [P, S], F16, kind="ExternalInput")
    sinP = nc.dram_tensor("sinP", [P, S], F16, kind="ExternalInput")
    ident = nc.dram_tensor("ident", [P, P], F16, kind="ExternalInput")
    umask = nc.dram_tensor("umask", [P, 512], F16, kind="ExternalInput")
    negI = nc.dram_tensor("negI", [P, P], F16, kind="ExternalInput")
    umask8 = nc.dram_tensor("umask8", [P, 2, 512], F8, kind="ExternalInput")
    negI8 = nc.dram_tensor("negI8", [P, 2, P], F8, kind="ExternalInput")
    outT = nc.dram_tensor("outT", [D, S], F32, kind="ExternalOutput")
    if CFG.get("debug"):
        dQT = nc.dram_tensor("dQT", [P, 4, S], F16, kind="ExternalOutput")
        dKT = nc.dram_tensor("dKT", [P, 4, S], F16, kind="ExternalOutput")
        dVP = nc.dram_tensor("dVP", [P, 16, NH, DK + 1], F16, kind="ExternalOutput")
        dHOPT = nc.dram_tensor("dHOPT", [P, 4, S], F16, kind="ExternalOutput")
        CFG["_dbg"] = (dQT, dKT, dVP, dHOPT)

    hl = None
    if CFG["proj_hilo"]:
        hl = {}
        for nm in ("x8h", "x8l"):
            hl[nm] = nc.dram_tensor(nm, [P, D // 256, 2, S], F8, kind="ExternalInput")
        for nm in ("wq8h", "wq8l", "wk8h", "wk8l", "wv8h", "wv8l"):
            hl[nm] = nc.dram_tensor(nm, [P, D // 256, 2, HD], F8, kind="ExternalInput")

    with tile.TileContext(nc) as tc:
        if nrep == 1:
            _attn_tile(tc, xh, hl, wqh, wkh, wvh, woh, cosP, sinP,
                       ident, umask, negI, umask8, negI8, outT)
        else:
            with tc.For_i(0, nrep, 1):
                _attn_tile(tc, xh, hl, wqh, wkh, wvh, woh, cosP,
                           sinP, ident, umask, negI, umask8, negI8, outT)
    nc.compile()
    return nc


def _attn_tile(tc, xh, hl, wqh, wkh, wvh, woh, cosP, sinP, ident,
               umask, negI, umask8, negI8, outT):
    nc = tc.nc
    s_dr = CFG["s_dr"]
    hilo = CFG["proj_hilo"]
    need_f8 = s_dr != "none"
    need_f16qt = s_dr != "all"

    with (
        tc.tile_pool(name="per", bufs=1) as per,
        tc.tile_pool(name="ab", bufs=8) as abp,
        tc.tile_pool(name="rt", bufs=6) as rtp,
        tc.tile_pool(name="ptp", bufs=4) as ptp,
        tc.tile_pool(name="arp", bufs=6) as arp,
        tc.tile_pool(name="rcp", bufs=4) as rcp,
        tc.tile_pool(name="dgp", bufs=6) as dgp,
        tc.tile_pool(name="obp", bufs=2) as obp,
        tc.tile_pool(name="psS", bufs=2, space="PSUM") as psS,
        tc.tile_pool(name="psPV", bufs=2, space="PSUM") as psPV,
        tc.tile_pool(name="psC", bufs=2, space="PSUM") as psC,
    ):
        # ---------------- persistent tiles + input DMAs ----------------
        if need_f16qt:
            QT = per.tile([P, 4, S], F16, tag="QT")
            KT = per.tile([P, 4, S], F16, tag="KT")
        else:
            QT = KT = None
        if need_f8:
            QT8 = per.tile([P, 4, S], F8, tag="QT8")
            KT8 = per.tile([P, 4, S], F8, tag="KT8")
        else:
            QT8 = KT8 = None
        VP = per.tile([P, 16, NH, DK + 1], F16, tag="VP")
        HOPT = per.tile([P, 4, S], F16, tag="HOPT")

        # ones column of VP (col 0 per head)
        nc.vector.memset(VP[:, :, :, 0:1], 1.0)

        # DMAs ordered so phase_a(0, 0) can start as early as possible:
        # j-pair-0 halves of wq first, then the first x chunk, wk, cos/sin, wv.
        if hilo:
            hsb = {}
            for nm in ("x8h", "x8l"):
                hsb[nm] = per.tile([P, 4, 2, S], F8, tag=nm, name=nm)
            for nm in ("wq8h", "wq8l", "wk8h", "wk8l", "wv8h", "wv8l"):
                hsb[nm] = per.tile([P, 4, 2, HD], F8, tag=nm, name=nm)

            def hsrc(nm):
                return hl[nm].ap()

            for nm in ("wq8h", "wq8l"):
                nc.sync.dma_start(hsb[nm][:, :, :, 0:256], hsrc(nm)[:, :, :, 0:256])
            for nm in ("x8h", "x8l"):
                nc.sync.dma_start(hsb[nm][:, :, :, 0:512], hsrc(nm)[:, :, :, 0:512])
            for nm in ("wk8h", "wk8l"):
                nc.sync.dma_start(hsb[nm][:, :, :, 0:256], hsrc(nm)[:, :, :, 0:256])
            for nm in ("wv8h", "wv8l"):
                nc.sync.dma_start(hsb[nm][:, :, 0:1, :], hsrc(nm)[:, :, 0:1, :])
            cos_sb = per.tile([P, S], F16, tag="cos")
            sin_sb = per.tile([P, S], F16, tag="sin")
            nc.sync.dma_start(cos_sb, cosP.ap())
            nc.sync.dma_start(sin_sb, sinP.ap())
            for nm in ("wv8h", "wv8l"):
                nc.sync.dma_start(hsb[nm][:, :, 1:2, :], hsrc(nm)[:, :, 1:2, :])
            for nm in ("wq8h", "wq8l", "wk8h", "wk8l"):
                nc.sync.dma_start(hsb[nm][:, :, :, 256:512], hsrc(nm)[:, :, :, 256:512])
            for tci in range(1, 4):
                tsl = ds(tci * 512, 512)
                for nm in ("x8h", "x8l"):
                    nc.sync.dma_start(hsb[nm][:, :, :, tsl], hsrc(nm)[:, :, :, tsl])
        else:
            xh_sb = per.tile([P, 8, S], F16, tag="xh")
            xh_src = xh.ap().rearrange("o p t -> p o t")
            wq_sb = per.tile([P, 8, HD], F16, tag="wq")
            wk_sb = per.tile([P, 8, HD], F16, tag="wk")
            wq_src = wqh.ap().rearrange("o p d -> p o d")
            wk_src = wkh.ap().rearrange("o p d -> p o d")
            for jp in (0, 1):
                dsl = ds(jp * 256, 256)
                nc.sync.dma_start(wq_sb[:, :, dsl], wq_src[:, :, dsl])
                nc.sync.dma_start(wk_sb[:, :, dsl], wk_src[:, :, dsl])
            nc.sync.dma_start(xh_sb[:, :, 0:512], xh_src[:, :, 0:512])
            cos_sb = per.tile([P, S], F16, tag="cos")
            sin_sb = per.tile([P, S], F16, tag="sin")
            nc.sync.dma_start(cos_sb, cosP.ap())
            nc.sync.dma_start(sin_sb, sinP.ap())
            wv_sb = per.tile([P, 8, HD], F16, tag="wv")
            nc.sync.dma_start(wv_sb, wvh.ap().rearrange("o p d -> p o d"))
            for tci in range(1, 4):
                tsl = ds(tci * 512, 512)
                nc.sync.dma_start(xh_sb[:, :, tsl], xh_src[:, :, tsl])
        wo_sb = per.tile([P, 4, D], F16, tag="wo")
        nc.sync.dma_start(wo_sb, woh.ap().rearrange("m p o -> p m o"))
        I_sb = per.tile([P, P], F16, tag="I")
        nc.sync.dma_start(I_sb, ident.ap())
        z_sb = per.tile([P, 512], F16, tag="z")
        nc.vector.memset(z_sb[0:1, :], 0.0)
        um_sb = per.tile([P, 512], F16, tag="um")
        nc.sync.dma_start(um_sb, umask.ap())
        nI_sb = per.tile([P, P], F16, tag="nI")
        nc.sync.dma_start(nI_sb, negI.ap())
        if need_f8:
            um8_sb = per.tile([P, 2, 512], F8, tag="um8")
            nc.sync.dma_start(um8_sb, umask8.ap())
            nI8_sb = per.tile([P, 2, P], F8, tag="nI8")
            nc.sync.dma_start(nI8_sb, negI8.ap())

        outT_ap = outT.ap()

        def act_copy(out, in_):
            nc.scalar.activation(out, in_, mybir.ActivationFunctionType.Copy)

        # ---------------- phase A half-chunk (t block 512, one j-pair) ----
        def phase_a_units(tci, jp):
            """Yield small closures: 2 Q-proj tiles, ropeQ, 2 K-proj tiles,
            ropeK, 2 V tiles. Emitted piecemeal between attention kts."""
            tsl = ds(tci * 512, 512)
            abt = {}

            def proj_unit(wname, j):
              def go():
                    ps = psC.tile([P, 512], F32, tag="c", name=f"pa{tci}{j}")
                    if hilo:
                        first = True
                        for c in range(4):
                            for wsfx, xsfx in (("h", "h"), ("h", "l"), ("l", "h")):
                                nc.tensor.matmul(
                                    ps,
                                    lhsT=hsb[f"{wname}8{wsfx}"][:, c, :, ts(j, P)],
                                    rhs=hsb[f"x8{xsfx}"][:, c, :, tsl],
                                    start=first, stop=(c == 3 and wsfx == "l"),
                                    perf_mode=DR,
                                )
                                first = False
                    else:
                        w = wq_sb if wname == "wq" else wk_sb
                        for o in range(8):
                            nc.tensor.matmul(
                                ps,
                                lhsT=w[:, o, ts(j, P)],
                                rhs=xh_sb[:, o, tsl],
                                start=(o == 0), stop=(o == 7),
                            )
                    ab = abp.tile([P, 512], F16, tag="ab", name=f"ab{j}")
                    nc.vector.tensor_copy(ab, ps)
                    abt[j] = ab
              return go

            def v_unit(tt):
              def go():
                    psv = psC.tile([P, 512], F32, tag="c", name=f"pvv{tci}{tt}")
                    xsl = ds(tci * 512 + tt * P, P)
                    if hilo:
                        first = True
                        for c in range(4):
                            for xsfx, wsfx in (("h", "h"), ("h", "l"), ("l", "h")):
                                nc.tensor.matmul(
                                    psv,
                                    lhsT=hsb[f"x8{xsfx}"][:, c, :, xsl],
                                    rhs=hsb[f"wv8{wsfx}"][:, c, :, :],
                                    start=first, stop=(c == 3 and xsfx == "l"),
                                    perf_mode=DR,
                                )
                                first = False
                    else:
                        for o in range(8):
                            nc.tensor.matmul(
                                psv,
                                lhsT=xh_sb[:, o, xsl],
                                rhs=wv_sb[:, o, :],
                                start=(o == 0), stop=(o == 7),
                            )
                    nc.vector.tensor_copy(
                        VP[:, tci * 4 + tt, :, 1:],
                        psv.rearrange("p (h c) -> p h c", c=DK),
                    )
              return go

            def rope_unit(tensor):
              def go():
                    mul_eng = nc.gpsimd if CFG["rope_mul_pool"] else nc.vector
                    je, jo = 2 * jp, 2 * jp + 1
                    A, Bb = abt[je] if tensor == 0 else abt[je + 10], None
                    # resolved below
              return go

            yield proj_unit("wq", 2 * jp)
            yield proj_unit("wq", 2 * jp + 1)
            yield _rope_closure(tci, jp, abt, 0)
            yield proj_unit("wk", 2 * jp)
            yield proj_unit("wk", 2 * jp + 1)
            yield _rope_closure(tci, jp, abt, 1)
            yield v_unit(2 * jp)
            yield v_unit(2 * jp + 1)

        def _unused(tci, jp):
            tsl = ds(tci * 512, 512)
            abt = []
            for wname in ("wq", "wk"):
                for j in (2 * jp, 2 * jp + 1):
                    ps = psC.tile([P, 512], F32, tag="c", name=f"pa{tci}{j}")
                    if hilo:
                        first = True
                        for c in range(4):
                            for wsfx, xsfx in (("h", "h"), ("h", "l"), ("l", "h")):
                                nc.tensor.matmul(
                                    ps,
                                    lhsT=hsb[f"{wname}8{wsfx}"][:, c, :, ts(j, P)],
                                    rhs=hsb[f"x8{xsfx}"][:, c, :, tsl],
                                    start=first, stop=(c == 3 and wsfx == "l"),
                                    perf_mode=DR,
                                )
                                first = False
                    else:
                        w = wq_sb if wname == "wq" else wk_sb
                        for o in range(8):
                            nc.tensor.matmul(
                                ps,
                                lhsT=w[:, o, ts(j, P)],
                                rhs=xh_sb[:, o, tsl],
                                start=(o == 0), stop=(o == 7),
                            )
                    ab = abp.tile([P, 512], F16, tag="ab", name=f"ab{j}")
                    act_copy(ab, ps)
                    abt.append(ab)
            # V projection for two t-subtiles
            for tt in (2 * jp, 2 * jp + 1):
                psv = psC.tile([P, 512], F32, tag="c", name=f"pvv{tci}{tt}")
                xsl = ds(tci * 512 + tt * P, P)
                if hilo:
                    first = True
                    for c in range(4):
                        for xsfx, wsfx in (("h", "h"), ("h", "l"), ("l", "h")):
                            nc.tensor.matmul(
                                psv,
                                lhsT=hsb[f"x8{xsfx}"][:, c, :, xsl],
                                rhs=hsb[f"wv8{wsfx}"][:, c, :, :],
                                start=first, stop=(c == 3 and xsfx == "l"),
                                perf_mode=DR,
                            )
                            first = False
                else:
                    for o in range(8):
                        nc.tensor.matmul(
                            psv,
                            lhsT=xh_sb[:, o, xsl],
                            rhs=wv_sb[:, o, :],
                            start=(o == 0), stop=(o == 7),
                        )
                act_copy(
                    VP[:, tci * 4 + tt, :, 1:],
                    psv.rearrange("p (h c) -> p h c", c=DK),
                )
            # RoPE for this j-pair, both tensors
            mul_eng = nc.gpsimd if CFG["rope_mul_pool"] else nc.vector
            for tensor in (0, 1):
                A, Bb = abt[2 * tensor], abt[2 * tensor + 1]
                dst = (QT, KT)[tensor]
                dst8 = (QT8, KT8)[tensor]
                je, jo = 2 * jp, 2 * jp + 1
                t1 = rtp.tile([P, 512], F16, tag="rt", name="t1")
                t2 = rtp.tile([P, 512], F16, tag="rt", name="t2")
                mul_eng.tensor_mul(t1, cos_sb[:, tsl], A)
                mul_eng.tensor_mul(t2, sin_sb[:, tsl], Bb)
                t3 = rtp.tile([P, 512], F16, tag="rt", name="t3")
                t4 = rtp.tile([P, 512], F16, tag="rt", name="t4")
                mul_eng.tensor_mul(t3, sin_sb[:, tsl], A)
                mul_eng.tensor_mul(t4, cos_sb[:, tsl], Bb)
                if need_f16qt:
                    nc.vector.tensor_sub(dst[:, je, tsl], t1, t2)
                    nc.vector.tensor_add(dst[:, jo, tsl], t3, t4)
                    if need_f8:
                        nc.gpsimd.tensor_copy(dst8[:, je, tsl], dst[:, je, tsl])
                        nc.gpsimd.tensor_copy(dst8[:, jo, tsl], dst[:, jo, tsl])
                else:
                    nc.vector.tensor_sub(dst8[:, je, tsl], t1, t2)
                    nc.vector.tensor_add(dst8[:, jo, tsl], t3, t4)

        # ---------------- attention stages ----------------
        def kt_loop(qb, m):
            """S + exp + PV accumulation for head pair m of q block qb.
            Returns state consumed by normalize()."""
            use_dr = s_dr == "all" or (s_dr == "ge512" and qb >= 1)
            pvt = [
                psPV.tile([P, 2, 130], F32, tag="pv", name=f"pv{qb}{m}{i}")
                for i in range(2)
            ]
            # start=True zeroes the whole 2KB bank (ZERO_REGION_SIZE), so a
            # shared-bank tile is zeroed once up front and all sub-region
            # accumulations use start=False.
            for i in range(2):
                nc.tensor.matmul(
                    pvt[i], lhsT=z_sb[0:1, 0:P], rhs=z_sb[0:1, 0:260],
                    start=True, stop=False, skip_group_check=True,
                )
            nkt = 4 * qb + 4
            pend_pv = None  # (kt, pt2): PV deferred one kt so S(kt+1)
                            # precedes PV(kt) in the in-order PE queue

            def emit_pv(kt, pt2):
                roff_ = kt - 4 * qb
                for e in (0, 1):
                    h = 2 * m + e
                    for qt in range(4):
                        if qt < roff_:
                            continue
                        nc.tensor.matmul(
                            pvt[qt // 2][:, qt % 2, ds(e * 65, 65)],
                            lhsT=pt2[:, e, ds(qt * P, P)],
                            rhs=VP[:, kt, h, :],
                            start=False,
                            stop=(kt == 4 * qb + qt),
                            skip_group_check=True,
                        )

            for kt in range(nkt):
                roff = kt - 4 * qb
                c0 = P * max(roff, 0)
                live = 512 - c0
                s2 = psS.tile([P, 2, 512], F32, tag="s", name=f"s{qb}{m}{kt}")
                for e in (0, 1):
                    h = 2 * m + e
                    g, h4 = h // 4, h % 4
                    pr = ds(h4 * 32, 32)
                    tpos = (h4 * 32, 0)
                    if use_dr:
                        nc.tensor.matmul(
                            s2[:, e, c0:],
                            lhsT=KT8[pr, ds(2 * g, 2), ts(kt, P)],
                            rhs=QT8[pr, ds(2 * g, 2), ds(qb * 512 + c0, live)],
                            start=True, stop=(roff < 0),
                            perf_mode=DR,
                            skip_group_check=True,
                            tile_position=tpos,
                        )
                    else:
                        nc.tensor.matmul(
                            s2[:, e, c0:],
                            lhsT=KT[pr, 2 * g, ts(kt, P)],
                            rhs=QT[pr, 2 * g, ds(qb * 512 + c0, live)],
                            start=True, stop=False,
                            skip_group_check=True,
                            tile_position=tpos,
                        )
                        nc.tensor.matmul(
                            s2[:, e, c0:],
                            lhsT=KT[pr, 2 * g + 1, ts(kt, P)],
                            rhs=QT[pr, 2 * g + 1, ds(qb * 512 + c0, live)],
                            start=False, stop=(roff < 0),
                            skip_group_check=True,
                            tile_position=tpos,
                        )
                    if roff >= 0:
                        if use_dr:
                            nc.tensor.matmul(
                                s2[:, e, ds(c0, P)],
                                lhsT=nI8_sb,
                                rhs=um8_sb[:, :, 0:P],
                                start=False, stop=True,
                                perf_mode=DR,
                                skip_group_check=True,
                            )
                        else:
                            nc.tensor.matmul(
                                s2[:, e, ds(c0, P)],
                                lhsT=nI_sb,
                                rhs=um_sb[:, 0:P],
                                start=False, stop=True,
                                skip_group_check=True,
                            )
                pt2 = ptp.tile([P, 2, 512], F16, tag="pt")
                nc.scalar.activation(
                    pt2[:, :, c0:], s2[:, :, c0:],
                    mybir.ActivationFunctionType.Exp, scale=0.125,
                )
                if pend_pv is not None:
                    emit_pv(*pend_pv)
                pend_pv = (kt, pt2)
                drain(1)
            emit_pv(*pend_pv)
            return pvt

        def norm_a(qb, m, pvt):
            """Drain pv psum right away: recip denominators to SBUF + raw
            fp16 copy. DVE-only, so the PE queue is not blocked; frees the
            pv psum ring for the next kt_loop."""
            rc = rcp.tile([P, 2, 2, 2], F32, tag="rc", name=f"rc{qb}{m}")
            araw = []
            for i in (0, 1):
                rsl = pvt[i].rearrange("p a (b c) -> p a b c", c=65)[:, :, :, 0]
                nc.vector.reciprocal(rc[:, i], rsl)
                ar = arp.tile([P, 2, 130], F16, tag="ar", name=f"ar{i}")
                if hilo:
                    nc.vector.tensor_scalar_mul(ar, pvt[i], 1.0 / WSC)
                else:
                    nc.vector.tensor_copy(ar, pvt[i])
                araw.append(ar)
            return rc, araw

        def norm_b_units(qb, m, rc, araw):
            """Deferred: diag builds + scaled transpose into HOPT, split into
            two filler units so the PE queue never blocks long on them."""
            qsl = ds(qb * 512, 512)
            st = {}

            def half(which):
                def go():
                    if which == 0:
                        st["tp"] = psC.tile([P, 512], F32, tag="c",
                                            name=f"tp{qb}{m}")
                        nc.tensor.matmul(
                            st["tp"], lhsT=z_sb[0:1, 0:P], rhs=z_sb[0:1, :],
                            start=True, stop=False, skip_group_check=True,
                        )
                    tp = st["tp"]
                    for qt in (0, 1) if which == 0 else (2, 3):
                        for e in (0, 1):
                            dg = dgp.tile([P, P], F16, tag="dg")
                            nc.vector.tensor_scalar_mul(
                                dg, I_sb, rc[:, qt // 2, qt % 2, ds(e, 1)]
                            )
                            nc.tensor.matmul(
                                tp[ds(e * DK, DK), ts(qt, P)],
                                lhsT=araw[qt // 2][:, qt % 2, ds(e * 65 + 1, DK)],
                                rhs=dg,
                                start=False, stop=(qt == 3),
                                skip_group_check=True,
                            )
                    if which == 1:
                        nc.vector.tensor_copy(HOPT[:, m, qsl], tp)
                return go

            return [half(0), half(1)]

        def norm_b(qb, m, rc, araw):
            for u in norm_b_units(qb, m, rc, araw):
                u()

        def o_proj(qb):
            qsl = ds(qb * 512, 512)
            for og in range(2):
                ob = obp.tile([P, 4, 512], F32, tag="ob")
                for oi in range(4):
                    ot = og * 4 + oi
                    psc = psC.tile([P, 512], F32, tag="c", name=f"oc{qb}{ot}")
                    for mm in range(4):
                        nc.tensor.matmul(
                            psc,
                            lhsT=wo_sb[:, mm, ts(ot, P)],
                            rhs=HOPT[:, mm, qsl],
                            start=(mm == 0), stop=(mm == 3),
                        )
                    nc.vector.tensor_copy(ob[:, oi, :], psc)
                nc.sync.dma_start(
                    outT_ap[ds(og * 512, 512), qsl].rearrange(
                        "(o p) q -> p o q", p=P
                    ),
                    ob,
                )

        # ---------------- emission schedule (software-pipelined) ----------
        # Filler units (phase-A pieces, o_proj pieces) are drained one per kt
        # inside the attention loops so the PE always interleaves projection
        # work with the exp-feeding S-matmuls instead of blocking Act behind
        # multi-microsecond projection bursts.
        import os
        from collections import deque

        filler = deque()  # (tag, closure); tag = ("a", tci) or ("o", qb)

        def drain(n=1):
            for _ in range(n):
                if filler:
                    filler.popleft()[1]()

        def flush_a(tci):
            while any(t[0] == "a" and t[1] <= tci for t, _ in filler):
                filler.popleft()[1]()

        def o_units(qb):
            qsl = ds(qb * 512, 512)
            units = []
            obt = {}

            def ou(og, oi):
                def go():
                    if oi == 0:
                        obt[og] = obp.tile([P, 4, 512], F32, tag="ob",
                                           name=f"ob{qb}{og}")
                    ot = og * 4 + oi
                    psc = psC.tile([P, 512], F32, tag="c", name=f"oc{qb}{ot}")
                    for mm in range(4):
                        nc.tensor.matmul(
                            psc,
                            lhsT=wo_sb[:, mm, ts(ot, P)],
                            rhs=HOPT[:, mm, qsl],
                            start=(mm == 0), stop=(mm == 3),
                        )
                    nc.vector.tensor_copy(obt[og][:, oi, :], psc)
                    if oi % 2 == 1:
                        nc.sync.dma_start(
                            outT_ap[ds(og * 512 + (oi - 1) * P, 256), qsl].rearrange(
                                "(o p) q -> p o q", p=P
                            ),
                            obt[og][:, ds(oi - 1, 2), :],
                        )
                return go

            for og in range(2):
                for oi in range(4):
                    units.append(ou(og, oi))
            return units

        phase_a(0, 0)
        phase_a(0, 1)

        push_at = {
            (0, 0): [("a", 1)],
            (1, 0): [("a", 2)],
            (2, 0): [("a", 3), ("o", 0)],
            (3, 0): [("o", 1)],
            (3, 2): [("o", 2)],
        }

        pend_norm = None
        for qb in range(4):
            for m in range(4):
                for kind, idx in push_at.get((qb, m), []):
                    if kind == "o":
                        while any(t[0] == "n" and t[1] == idx for t, _ in filler):
                            filler.popleft()[1]()
                    if kind == "a":
                        for jp in (0, 1):
                            for u in phase_a_units(idx, jp):
                                filler.append((("a", idx), u))
                    else:
                        for u in o_units(idx):
                            filler.append((("o", idx), u))
                if m == 0:
                    flush_a(qb)
                pvt = kt_loop(qb, m)
                rc, araw = norm_a(qb, m, pvt)
                if pend_norm is not None:
                    nq, nm, nrc, nar = pend_norm
                    for u in reversed(norm_b_units(nq, nm, nrc, nar)):
                        filler.appendleft((("n", nq), u))
                pend_norm = (qb, m, rc, araw)
        while filler:
            filler.popleft()[1]()
        norm_b(*pend_norm)
        for u in o_units(3):
            u()

        if CFG.get("debug"):
            dQT, dKT, dVP, dHOPT = CFG["_dbg"]
            nc.sync.dma_start(dQT.ap(), QT if need_f16qt else QT8)
            nc.sync.dma_start(dKT.ap(), KT if need_f16qt else KT8)
            nc.sync.dma_start(dHOPT.ap(), HOPT)
            nc.sync.dma_start(dVP.ap(), VP)


# ---------------- host side ----------------

def _qk_perm():
    perm = []
    for j in range(4):
        for h4 in range(4):
            h = (j // 2) * 4 + h4
            par = j % 2
            perm.extend(h * 64 + 2 * np.arange(32) + par)
    return np.array(perm)


_PERM = _qk_perm()


def _host_tables():
    import ml_dtypes

    F16n = np.float16
    i = np.arange(32, dtype=np.float32)
    inv_freq = (THETA ** (2.0 * i / DK)).astype(np.float32)
    t = np.arange(S, dtype=np.float32)
    ang = t[None, :] / inv_freq[:, None]          # [32, S]
    sc = 1.0 / WSC if CFG["proj_hilo"] else 1.0
    cosP = np.tile(sc * np.cos(ang), (4, 1)).astype(F16n)   # [128, S]
    sinP = np.tile(sc * np.sin(ang), (4, 1)).astype(F16n)
    ident = np.eye(P, dtype=F16n)
    kk = np.arange(P)[:, None]
    qq = np.arange(512)[None, :]
    umask = (kk > qq).astype(F16n)                # [128, 512]
    negI = (NEG * np.eye(P)).astype(F16n)
    F8n = ml_dtypes.float8_e4m3fn
    negI8 = np.zeros((P, 2, P), dtype=np.float32)
    umask8 = np.zeros((P, 2, 512), dtype=np.float32)
    for ii in range(2):
        for p in range(64):
            negI8[p, ii, ii * 64 + p] = NEG8
            umask8[p, ii, :] = ((ii * 64 + p) > qq[0]).astype(np.float32)
    return cosP, sinP, ident, umask, negI, umask8.astype(F8n), negI8.astype(F8n)


def make_in_maps(x, Wq, Wk, Wv, Wo):
    import ml_dtypes

    F16n = np.float16
    F8n = ml_dtypes.float8_e4m3fn
    cosP, sinP, ident, umask, negI, umask8, negI8 = _host_tables()
    in_maps = []
    for c in range(8):
        b, hh = c // 2, c % 2
        sl = slice(hh * HD, (hh + 1) * HD)
        xT = np.ascontiguousarray(x[b].T)                      # [1024, 2048]
        wq = Wq[sl, :][_PERM]
        wk = Wk[sl, :][_PERM]
        wv = Wv[sl, :]
        wo = Wo[:, sl]                                         # [1024, 512]
        woh = np.ascontiguousarray(wo.T).reshape(4, P, D)
        m = {
            "xh": xT.reshape(8, P, S).astype(F16n),
            "wqh": np.ascontiguousarray(wq.T).reshape(8, P, HD).astype(F16n),
            "wkh": np.ascontiguousarray(wk.T).reshape(8, P, HD).astype(F16n),
            "wvh": np.ascontiguousarray(wv.T).reshape(8, P, HD).astype(F16n),
            "woh": woh.astype(F16n),
            "cosP": cosP, "sinP": sinP, "ident": ident,
            "umask": umask, "negI": negI, "umask8": umask8, "negI8": negI8,
        }
        if CFG["proj_hilo"]:
            def drfmt(a, n):
                return np.ascontiguousarray(a).reshape(4, 2, P, n).transpose(2, 0, 1, 3)

            def hilo8(a, n, scale):
                a = drfmt(a, n) * scale
                hi = a.astype(F8n)
                lo = (a - hi.astype(np.float32)).astype(F8n)
                return hi, lo

            m["x8h"], m["x8l"] = hilo8(xT, S, 1.0)
            m["wq8h"], m["wq8l"] = hilo8(wq.T, HD, WSC)
            m["wk8h"], m["wk8l"] = hilo8(wk.T, HD, WSC)
            m["wv8h"], m["wv8l"] = hilo8(wv.T, HD, WSC)
        in_maps.append(m)
    return in_maps


def gather_out(core_outs):
    out = np.empty((B, S, D), dtype=np.float32)
    for b in range(B):
        out[b] = (core_outs[2 * b]["outT"] + core_outs[2 * b + 1]["outT"]).T
    return out


_NC_CACHE = {}


def kernel(x, Wq, Wk, Wv, Wo):
    x = np.asarray(x, dtype=np.float32)
    Wq = np.asarray(Wq, dtype=np.float32)
    Wk = np.asarray(Wk, dtype=np.float32)
    Wv = np.asarray(Wv, dtype=np.float32)
    Wo = np.asarray(Wo, dtype=np.float32)
    if "nc" not in _NC_CACHE:
        _NC_CACHE["nc"] = build_attention_nc()
    nc = _NC_CACHE["nc"]
    in_maps = make_in_maps(x, Wq, Wk, Wv, Wo)
    res = run_bass_kernel_spmd(nc, in_maps, core_ids=list(range(8)))
    return gather_out(res.results)


# revision 7
# speedup vs baseline: 1.1673x; 1.0000x over previous
"""Causal MHA (B=4, S=2048, D=1024, H=16, RoPE) on 8 trn2 cores — v2.

Sharding: core c -> batch c//2, head-half c%2 (8 heads / 512 dims per core).

Design vs v1 baseline:
  - Q/K weights host-permuted into even/odd 32-blocks per 4-head group so
    RoPE needs NO partition-swap DMA: psum tile pairs (j, j+1) hold the
    even/odd halves at identical partition indices and the rotation is plain
    elementwise tensor ops (fp16, 2x DVE rate).
  - fp16 downstream (P, V, attn, Wo): 1 cyc/row matmuls.
  - Optional fp8e4m3 DoubleRow scores (0.5 cyc/row): the even/odd layout is
    natively DR-compatible ([32 parts, 2 j-slots, t] APs), no re-layout DMA.
  - PV in [q, dv] orientation (moving dim 65): psum accumulators
    [128, 2qt, 130] with ones-column densities; normalization via
    per-partition recips + diag-matmul transpose back to [d', t].
  - Software-pipelined emission: normalize(m) deferred into (m+1)'s kt loop,
    O-proj(qb) into qb+1, phase-A chunks interleaved at m boundaries, so the
    in-order PE queue never head-of-line blocks on DVE/Act chains.
"""

import numpy as np

import concourse.bass as bass
import concourse.bacc as bacc
import concourse.mybir as mybir
import concourse.tile as tile
from concourse.bass import ds, ts
from concourse.bass_utils import run_bass_kernel_spmd

F32 = mybir.dt.float32
F16 = mybir.dt.float16
F8 = mybir.dt.float8e4
DR = mybir.MatmulPerfMode.DoubleRow

B, S, D, H, DK = 4, 2048, 1024, 16, 64
THETA = 10000.0
NH = 8
HD = NH * DK
P = 128
NEG = -28000.0
NEG8 = -240.0

CFG = {
    "s_dr": "ge512",   # "none" | "ge512" | "all"
    "proj_dr": False,
    "proj_hilo": True,  # hi+lo fp8e4m3 DoubleRow projections (W pre-scaled x16)
    "rope_mul_pool": False,
}
WSC = 16.0  # weight pre-scale for hi/lo fp8 (folded out via rope tables / araw)


def build_attention_nc(nrep=1):
    nc = bacc.Bacc("TRN2", target_bir_lowering=False, debug=False)

    xh = nc.dram_tensor("xh", [D // P, P, S], F16, kind="ExternalInput")
    wqh = nc.dram_tensor("wqh", [D // P, P, HD], F16, kind="ExternalInput")
    wkh = nc.dram_tensor("wkh", [D // P, P, HD], F16, kind="ExternalInput")
    wvh = nc.dram_tensor("wvh", [D // P, P, HD], F16, kind="ExternalInput")
    woh = nc.dram_tensor("woh", [HD // P, P, D], F16, kind="ExternalInput")
    cosP = nc.dram_tensor("cosP", [P, S], F16, kind="ExternalInput")
    sinP = nc.dram_tensor("sinP", [P, S], F16, kind="ExternalInput")
    ident = nc.dram_tensor("ident", [P, P], F16, kind="ExternalInput")
    umask = nc.dram_tensor("umask", [P, 512], F16, kind="ExternalInput")
    negI = nc.dram_tensor("negI", [P, P], F16, kind="ExternalInput")
    umask8 = nc.dram_tensor("umask8", [P, 2, 512], F8, kind="ExternalInput")
    negI8 = nc.dram_tensor("negI8", [P, 2, P], F8, kind="ExternalInput")
    outT = nc.dram_tensor("outT", [D, S], F32, kind="ExternalOutput")
    if CFG.get("debug"):
        dQT = nc.dram_tensor("dQT", [P, 4, S], F16, kind="ExternalOutput")
        dKT = nc.dram_tensor("dKT", [P, 4, S], F16, kind="ExternalOutput")
        dVP = nc.dram_tensor("dVP", [P, 16, NH, DK + 1], F16, kind="ExternalOutput")
        dHOPT = nc.dram_tensor("dHOPT", [P, 4, S], F16, kind="ExternalOutput")
        CFG["_dbg"] = (dQT, dKT, dVP, dHOPT)

    hl = None
    if CFG["proj_hilo"]:
        hl = {}
        for nm in ("x8h", "x8l"):
            hl[nm] = nc.dram_tensor(nm, [P, D // 256, 2, S], F8, kind="ExternalInput")
        for nm in ("wq8h", "wq8l", "wk8h", "wk8l", "wv8h", "wv8l"):
            hl[nm] = nc.dram_tensor(nm, [P, D // 256, 2, HD], F8, kind="ExternalInput")

    with tile.TileContext(nc) as tc:
        if nrep == 1:
            _attn_tile(tc, xh, hl, wqh, wkh, wvh, woh, cosP, sinP,
                       ident, umask, negI, umask8, negI8, outT)
        else:
            with tc.For_i(0, nrep, 1):
                _attn_tile(tc, xh, hl, wqh, wkh, wvh, woh, cosP,
                           sinP, ident, umask, negI, umask8, negI8, outT)
    nc.compile()
    return nc


def _attn_tile(tc, xh, hl, wqh, wkh, wvh, woh, cosP, sinP, ident,
               umask, negI, umask8, negI8, outT):
    nc = tc.nc
    s_dr = CFG["s_dr"]
    hilo = CFG["proj_hilo"]
    need_f8 = s_dr != "none"
    need_f16qt = s_dr != "all"

    with (
        tc.tile_pool(name="per", bufs=1) as per,
        tc.tile_pool(name="ab", bufs=8) as abp,
        tc.tile_pool(name="rt", bufs=6) as rtp,
        tc.tile_pool(name="ptp", bufs=4) as ptp,
        tc.tile_pool(name="arp", bufs=6) as arp,
        tc.tile_pool(name="rcp", bufs=4) as rcp,
        tc.tile_pool(name="dgp", bufs=6) as dgp,
        tc.tile_pool(name="obp", bufs=2) as obp,
        tc.tile_pool(name="psS", bufs=2, space="PSUM") as psS,
        tc.tile_pool(name="psPV", bufs=2, space="PSUM") as psPV,
        tc.tile_pool(name="psC", bufs=2, space="PSUM") as psC,
    ):
        # ---------------- persistent tiles + input DMAs ----------------
        if need_f16qt:
            QT = per.tile([P, 4, S], F16, tag="QT")
            KT = per.tile([P, 4, S], F16, tag="KT")
        else:
            QT = KT = None
        if need_f8:
            QT8 = per.tile([P, 4, S], F8, tag="QT8")
            KT8 = per.tile([P, 4, S], F8, tag="KT8")
        else:
            QT8 = KT8 = None
        VP = per.tile([P, 16, NH, DK + 1], F16, tag="VP")
        HOPT = per.tile([P, 4, S], F16, tag="HOPT")

        # ones column of VP (col 0 per head)
        nc.vector.memset(VP[:, :, :, 0:1], 1.0)

        # DMAs ordered so phase_a(0, 0) can start as early as possible:
        # j-pair-0 halves of wq first, then the first x chunk, wk, cos/sin, wv.
        if hilo:
            hsb = {}
            for nm in ("x8h", "x8l"):
                hsb[nm] = per.tile([P, 4, 2, S], F8, tag=nm, name=nm)
            for nm in ("wq8h", "wq8l", "wk8h", "wk8l", "wv8h", "wv8l"):
                hsb[nm] = per.tile([P, 4, 2, HD], F8, tag=nm, name=nm)

            def hsrc(nm):
                return hl[nm].ap()

            for nm in ("wq8h", "wq8l"):
                nc.sync.dma_start(hsb[nm][:, :, :, 0:256], hsrc(nm)[:, :, :, 0:256])
            for nm in ("x8h", "x8l"):
                nc.sync.dma_start(hsb[nm][:, :, :, 0:512], hsrc(nm)[:, :, :, 0:512])
            for nm in ("wk8h", "wk8l"):
                nc.sync.dma_start(hsb[nm][:, :, :, 0:256], hsrc(nm)[:, :, :, 0:256])
            for nm in ("wv8h", "wv8l"):
                nc.sync.dma_start(hsb[nm][:, :, 0:1, :], hsrc(nm)[:, :, 0:1, :])
            cos_sb = per.tile([P, S], F16, tag="cos")
            sin_sb = per.tile([P, S], F16, tag="sin")
            nc.sync.dma_start(cos_sb, cosP.ap())
            nc.sync.dma_start(sin_sb, sinP.ap())
            for nm in ("wv8h", "wv8l"):
                nc.sync.dma_start(hsb[nm][:, :, 1:2, :], hsrc(nm)[:, :, 1:2, :])
            for nm in ("wq8h", "wq8l", "wk8h", "wk8l"):
                nc.sync.dma_start(hsb[nm][:, :, :, 256:512], hsrc(nm)[:, :, :, 256:512])
            for tci in range(1, 4):
                tsl = ds(tci * 512, 512)
                for nm in ("x8h", "x8l"):
                    nc.sync.dma_start(hsb[nm][:, :, :, tsl], hsrc(nm)[:, :, :, tsl])
        else:
            xh_sb = per.tile([P, 8, S], F16, tag="xh")
            xh_src = xh.ap().rearrange("o p t -> p o t")
            wq_sb = per.tile([P, 8, HD], F16, tag="wq")
            wk_sb = per.tile([P, 8, HD], F16, tag="wk")
            wq_src = wqh.ap().rearrange("o p d -> p o d")
            wk_src = wkh.ap().rearrange("o p d -> p o d")
            for jp in (0, 1):
                dsl = ds(jp * 256, 256)
                nc.sync.dma_start(wq_sb[:, :, dsl], wq_src[:, :, dsl])
                nc.sync.dma_start(wk_sb[:, :, dsl], wk_src[:, :, dsl])
            nc.sync.dma_start(xh_sb[:, :, 0:512], xh_src[:, :, 0:512])
            cos_sb = per.tile([P, S], F16, tag="cos")
            sin_sb = per.tile([P, S], F16, tag="sin")
            nc.sync.dma_start(cos_sb, cosP.ap())
            nc.sync.dma_start(sin_sb, sinP.ap())
            wv_sb = per.tile([P, 8, HD], F16, tag="wv")
            nc.sync.dma_start(wv_sb, wvh.ap().rearrange("o p d -> p o d"))
            for tci in range(1, 4):
                tsl = ds(tci * 512, 512)
                nc.sync.dma_start(xh_sb[:, :, tsl], xh_src[:, :, tsl])
        wo_sb = per.tile([P, 4, D], F16, tag="wo")
        nc.sync.dma_start(wo_sb, woh.ap().rearrange("m p o -> p m o"))
        I_sb = per.tile([P, P], F16, tag="I")
        nc.sync.dma_start(I_sb, ident.ap())
        z_sb = per.tile([P, 512], F16, tag="z")
        nc.vector.memset(z_sb[0:1, :], 0.0)
        um_sb = per.tile([P, 512], F16, tag="um")
        nc.sync.dma_start(um_sb, umask.ap())
        nI_sb = per.tile([P, P], F16, tag="nI")
        nc.sync.dma_start(nI_sb, negI.ap())
        if need_f8:
            um8_sb = per.tile([P, 2, 512], F8, tag="um8")
            nc.sync.dma_start(um8_sb, umask8.ap())
            nI8_sb = per.tile([P, 2, P], F8, tag="nI8")
            nc.sync.dma_start(nI8_sb, negI8.ap())

        outT_ap = outT.ap()

        def act_copy(out, in_):
            nc.scalar.activation(out, in_, mybir.ActivationFunctionType.Copy)

        # ---------------- phase A half-chunk (t block 512, one j-pair) ----
        def phase_a_units(tci, jp):
            """Yield small closures: 2 Q-proj tiles, ropeQ, 2 K-proj tiles,
            ropeK, 2 V tiles. Emitted piecemeal between attention kts."""
            tsl = ds(tci * 512, 512)
            abt = {}

            def proj_unit(wname, j):
              def go():
                    ps = psC.tile([P, 512], F32, tag="c", name=f"pa{tci}{j}")
                    if hilo:
                        first = True
                        for c in range(4):
                            for wsfx, xsfx in (("h", "h"), ("h", "l"), ("l", "h")):
                                nc.tensor.matmul(
                                    ps,
                                    lhsT=hsb[f"{wname}8{wsfx}"][:, c, :, ts(j, P)],
                                    rhs=hsb[f"x8{xsfx}"][:, c, :, tsl],
                                    start=first, stop=(c == 3 and wsfx == "l"),
                                    perf_mode=DR,
                                )
                                first = False
                    else:
                        w = wq_sb if wname == "wq" else wk_sb
                        for o in range(8):
                            nc.tensor.matmul(
                                ps,
                                lhsT=w[:, o, ts(j, P)],
                                rhs=xh_sb[:, o, tsl],
                                start=(o == 0), stop=(o == 7),
                            )
                    ab = abp.tile([P, 512], F16, tag="ab", name=f"ab{j}")
                    nc.vector.tensor_copy(ab, ps)
                    abt[j] = ab
              return go

            def v_unit(tt):
              def go():
                    psv = psC.tile([P, 512], F32, tag="c", name=f"pvv{tci}{tt}")
                    xsl = ds(tci * 512 + tt * P, P)
                    if hilo:
                        first = True
                        for c in range(4):
                            for xsfx, wsfx in (("h", "h"), ("h", "l"), ("l", "h")):
                                nc.tensor.matmul(
                                    psv,
                                    lhsT=hsb[f"x8{xsfx}"][:, c, :, xsl],
                                    rhs=hsb[f"wv8{wsfx}"][:, c, :, :],
                                    start=first, stop=(c == 3 and xsfx == "l"),
                                    perf_mode=DR,
                                )
                                first = False
                    else:
                        for o in range(8):
                            nc.tensor.matmul(
                                psv,
                                lhsT=xh_sb[:, o, xsl],
                                rhs=wv_sb[:, o, :],
                                start=(o == 0), stop=(o == 7),
                            )
                    nc.vector.tensor_copy(
                        VP[:, tci * 4 + tt, :, 1:],
                        psv.rearrange("p (h c) -> p h c", c=DK),
                    )
              return go

            def rope_unit(tensor):
              def go():
                    mul_eng = nc.gpsimd if CFG["rope_mul_pool"] else nc.vector
                    je, jo = 2 * jp, 2 * jp + 1
                    A, Bb = abt[je] if tensor == 0 else abt[je + 10], None
                    # resolved below
              return go

            yield proj_unit("wq", 2 * jp)
            yield proj_unit("wq", 2 * jp + 1)
            yield _rope_closure(tci, jp, abt, 0)
            yield proj_unit("wk", 2 * jp)
            yield proj_unit("wk", 2 * jp + 1)
            yield _rope_closure(tci, jp, abt, 1)
            yield v_unit(2 * jp)
            yield v_unit(2 * jp + 1)

        def _unused(tci, jp):
            tsl = ds(tci * 512, 512)
            abt = []
            for wname in ("wq", "wk"):
                for j in (2 * jp, 2 * jp + 1):
                    ps = psC.tile([P, 512], F32, tag="c", name=f"pa{tci}{j}")
                    if hilo:
                        first = True
                        for c in range(4):
                            for wsfx, xsfx in (("h", "h"), ("h", "l"), ("l", "h")):
                                nc.tensor.matmul(
                                    ps,
                                    lhsT=hsb[f"{wname}8{wsfx}"][:, c, :, ts(j, P)],
                                    rhs=hsb[f"x8{xsfx}"][:, c, :, tsl],
                                    start=first, stop=(c == 3 and wsfx == "l"),
                                    perf_mode=DR,
                                )
                                first = False
                    else:
                        w = wq_sb if wname == "wq" else wk_sb
                        for o in range(8):
                            nc.tensor.matmul(
                                ps,
                                lhsT=w[:, o, ts(j, P)],
                                rhs=xh_sb[:, o, tsl],
                                start=(o == 0), stop=(o == 7),
                            )
                    ab = abp.tile([P, 512], F16, tag="ab", name=f"ab{j}")
                    act_copy(ab, ps)
                    abt.append(ab)
            # V projection for two t-subtiles
            for tt in (2 * jp, 2 * jp + 1):
                psv = psC.tile([P, 512], F32, tag="c", name=f"pvv{tci}{tt}")
                xsl = ds(tci * 512 + tt * P, P)
                if hilo:
                    first = True
                    for c in range(4):
                        for xsfx, wsfx in (("h", "h"), ("h", "l"), ("l", "h")):
                            nc.tensor.matmul(
                                psv,
                                lhsT=hsb[f"x8{xsfx}"][:, c, :, xsl],
                                rhs=hsb[f"wv8{wsfx}"][:, c, :, :],
                                start=first, stop=(c == 3 and xsfx == "l"),
                                perf_mode=DR,
                            )
                            first = False
                else:
                    for o in range(8):
                        nc.tensor.matmul(
                            psv,
                            lhsT=xh_sb[:, o, xsl],
                            rhs=wv_sb[:, o, :],
                            start=(o == 0), stop=(o == 7),
                        )
                act_copy(
                    VP[:, tci * 4 + tt, :, 1:],
                    psv.rearrange("p (h c) -> p h c", c=DK),
                )
            # RoPE for this j-pair, both tensors
            mul_eng = nc.gpsimd if CFG["rope_mul_pool"] else nc.vector
            for tensor in (0, 1):
                A, Bb = abt[2 * tensor], abt[2 * tensor + 1]
                dst = (QT, KT)[tensor]
                dst8 = (QT8, KT8)[tensor]
                je, jo = 2 * jp, 2 * jp + 1
                t1 = rtp.tile([P, 512], F16, tag="rt", name="t1")
                t2 = rtp.tile([P, 512], F16, tag="rt", name="t2")
                mul_eng.tensor_mul(t1, cos_sb[:, tsl], A)
                mul_eng.tensor_mul(t2, sin_sb[:, tsl], Bb)
                t3 = rtp.tile([P, 512], F16, tag="rt", name="t3")
                t4 = rtp.tile([P, 512], F16, tag="rt", name="t4")
                mul_eng.tensor_mul(t3, sin_sb[:, tsl], A)
                mul_eng.tensor_mul(t4, cos_sb[:, tsl], Bb)
                if need_f16qt:
                    nc.vector.tensor_sub(dst[:, je, tsl], t1, t2)
                    nc.vector.tensor_add(dst[:, jo, tsl], t3, t4)
                    if need_f8:
                        nc.gpsimd.tensor_copy(dst8[:, je, tsl], dst[:, je, tsl])
                        nc.gpsimd.tensor_copy(dst8[:, jo, tsl], dst[:, jo, tsl])
                else:
                    nc.vector.tensor_sub(dst8[:, je, tsl], t1, t2)
                    nc.vector.tensor_add(dst8[:, jo, tsl], t3, t4)

        # ---------------- attention stages ----------------
        def kt_loop(qb, m):
            """S + exp + PV accumulation for head pair m of q block qb.
            Returns state consumed by normalize()."""
            use_dr = s_dr == "all" or (s_dr == "ge512" and qb >= 1)
            pvt = [
                psPV.tile([P, 2, 130], F32, tag="pv", name=f"pv{qb}{m}{i}")
                for i in range(2)
            ]
            # start=True zeroes the whole 2KB bank (ZERO_REGION_SIZE), so a
            # shared-bank tile is zeroed once up front and all sub-region
            # accumulations use start=False.
            for i in range(2):
                nc.tensor.matmul(
                    pvt[i][:, 0:1, 0:1], lhsT=z_sb[0:1, 0:P], rhs=z_sb[0:1, 0:1],
                    start=True, stop=False, skip_group_check=True,
                )
            nkt = 4 * qb + 4
            pend_pv = None  # (kt, pt2): PV deferred one kt so S(kt+1)
                            # precedes PV(kt) in the in-order PE queue

            def emit_pv(kt, pt2):
                roff_ = kt - 4 * qb
                for e in (0, 1):
                    h = 2 * m + e
                    for qt in range(4):
                        if qt < roff_:
                            continue
                        nc.tensor.matmul(
                            pvt[qt // 2][:, qt % 2, ds(e * 65, 65)],
                            lhsT=pt2[:, e, ds(qt * P, P)],
                            rhs=VP[:, kt, h, :],
                            start=False,
                            stop=(kt == 4 * qb + qt),
                            skip_group_check=True,
                        )

            for kt in range(nkt):
                roff = kt - 4 * qb
                c0 = P * max(roff, 0)
                live = 512 - c0
                s2 = psS.tile([P, 2, 512], F32, tag="s", name=f"s{qb}{m}{kt}")
                for e in (0, 1):
                    h = 2 * m + e
                    g, h4 = h // 4, h % 4
                    pr = ds(h4 * 32, 32)
                    tpos = (h4 * 32, 0)
                    if use_dr:
                        nc.tensor.matmul(
                            s2[:, e, c0:],
                            lhsT=KT8[pr, ds(2 * g, 2), ts(kt, P)],
                            rhs=QT8[pr, ds(2 * g, 2), ds(qb * 512 + c0, live)],
                            start=True, stop=(roff < 0),
                            perf_mode=DR,
                            skip_group_check=True,
                            tile_position=tpos,
                        )
                    else:
                        nc.tensor.matmul(
                            s2[:, e, c0:],
                            lhsT=KT[pr, 2 * g, ts(kt, P)],
                            rhs=QT[pr, 2 * g, ds(qb * 512 + c0, live)],
                            start=True, stop=False,
                            skip_group_check=True,
                            tile_position=tpos,
                        )
                        nc.tensor.matmul(
                            s2[:, e, c0:],
                            lhsT=KT[pr, 2 * g + 1, ts(kt, P)],
                            rhs=QT[pr, 2 * g + 1, ds(qb * 512 + c0, live)],
                            start=False, stop=(roff < 0),
                            skip_group_check=True,
                            tile_position=tpos,
                        )
                    if roff >= 0:
                        if use_dr:
                            nc.tensor.matmul(
                                s2[:, e, ds(c0, P)],
                                lhsT=nI8_sb,
                                rhs=um8_sb[:, :, 0:P],
                                start=False, stop=True,
                                perf_mode=DR,
                                skip_group_check=True,
                            )
                        else:
                            nc.tensor.matmul(
                                s2[:, e, ds(c0, P)],
                                lhsT=nI_sb,
                                rhs=um_sb[:, 0:P],
                                start=False, stop=True,
                                skip_group_check=True,
                            )
                pt2 = ptp.tile([P, 2, 512], F16, tag="pt")
                nc.scalar.activation(
                    pt2[:, :, c0:], s2[:, :, c0:],
                    mybir.ActivationFunctionType.Exp, scale=0.125,
                )
                if pend_pv is not None:
                    emit_pv(*pend_pv)
                pend_pv = (kt, pt2)
                drain(1)
            emit_pv(*pend_pv)
            return pvt

        def norm_a(qb, m, pvt):
            """Drain pv psum right away: recip denominators to SBUF + raw
            fp16 copy. DVE-only, so the PE queue is not blocked; frees the
            pv psum ring for the next kt_loop."""
            rc = rcp.tile([P, 2, 2, 2], F32, tag="rc", name=f"rc{qb}{m}")
            araw = []
            for i in (0, 1):
                rsl = pvt[i].rearrange("p a (b c) -> p a b c", c=65)[:, :, :, 0]
                nc.vector.reciprocal(rc[:, i], rsl)
                ar = arp.tile([P, 2, 130], F16, tag="ar", name=f"ar{i}")
                if hilo:
                    nc.vector.tensor_scalar_mul(ar, pvt[i], 1.0 / WSC)
                else:
                    nc.vector.tensor_copy(ar, pvt[i])
                araw.append(ar)
            return rc, araw

        def norm_b_units(qb, m, rc, araw):
            """Deferred: diag builds + scaled transpose into HOPT, split into
            two filler units so the PE queue never blocks long on them."""
            qsl = ds(qb * 512, 512)
            st = {}

            def half(which):
                def go():
                    if which == 0:
                        st["tp"] = psC.tile([P, 512], F32, tag="c",
                                            name=f"tp{qb}{m}")
                        nc.tensor.matmul(
                            st["tp"][:, 0:1], lhsT=z_sb[0:1, 0:P], rhs=z_sb[0:1, 0:1],
                            start=True, stop=False, skip_group_check=True,
                        )
                    tp = st["tp"]
                    for qt in (0, 1) if which == 0 else (2, 3):
                        for e in (0, 1):
                            dg = dgp.tile([P, P], F16, tag="dg")
                            nc.vector.tensor_scalar_mul(
                                dg, I_sb, rc[:, qt // 2, qt % 2, ds(e, 1)]
                            )
                            nc.tensor.matmul(
                                tp[ds(e * DK, DK), ts(qt, P)],
                                lhsT=araw[qt // 2][:, qt % 2, ds(e * 65 + 1, DK)],
                                rhs=dg,
                                start=False, stop=(qt == 3),
                                skip_group_check=True,
                            )
                    if which == 1:
                        nc.vector.tensor_copy(HOPT[:, m, qsl], tp)
                return go

            return [half(0), half(1)]

        def norm_b(qb, m, rc, araw):
            for u in norm_b_units(qb, m, rc, araw):
                u()

        def o_proj(qb):
            qsl = ds(qb * 512, 512)
            for og in range(2):
                ob = obp.tile([P, 4, 512], F32, tag="ob")
                for oi in range(4):
                    ot = og * 4 + oi
                    psc = psC.tile([P, 512], F32, tag="c", name=f"oc{qb}{ot}")
                    for mm in range(4):
                        nc.tensor.matmul(
                            psc,
                            lhsT=wo_sb[:, mm, ts(ot, P)],
                            rhs=HOPT[:, mm, qsl],
                            start=(mm == 0), stop=(mm == 3),
                        )
                    nc.vector.tensor_copy(ob[:, oi, :], psc)
                nc.sync.dma_start(
                    outT_ap[ds(og * 512, 512), qsl].rearrange(
                        "(o p) q -> p o q", p=P
                    ),
                    ob,
                )

        # ---------------- emission schedule (software-pipelined) ----------
        # Filler units (phase-A pieces, o_proj pieces) are drained one per kt
        # inside the attention loops so the PE always interleaves projection
        # work with the exp-feeding S-matmuls instead of blocking Act behind
        # multi-microsecond projection bursts.
        import os
        from collections import deque

        filler = deque()  # (tag, closure); tag = ("a", tci) or ("o", qb)

        def drain(n=1):
            for _ in range(n):
                if filler:
                    filler.popleft()[1]()

        def flush_a(tci):
            while any(t[0] == "a" and t[1] <= tci for t, _ in filler):
                filler.popleft()[1]()

        def o_units(qb):
            qsl = ds(qb * 512, 512)
            units = []
            obt = {}

            def ou(og, oi):
                def go():
                    if oi == 0:
                        obt[og] = obp.tile([P, 4, 512], F32, tag="ob",
                                           name=f"ob{qb}{og}")
                    ot = og * 4 + oi
                    psc = psC.tile([P, 512], F32, tag="c", name=f"oc{qb}{ot}")
                    for mm in range(4):
                        nc.tensor.matmul(
                            psc,
                            lhsT=wo_sb[:, mm, ts(ot, P)],
                            rhs=HOPT[:, mm, qsl],
                            start=(mm == 0), stop=(mm == 3),
                        )
                    nc.vector.tensor_copy(obt[og][:, oi, :], psc)
                    if oi % 2 == 1:
                        nc.sync.dma_start(
                            outT_ap[ds(og * 512 + (oi - 1) * P, 256), qsl].rearrange(
                                "(o p) q -> p o q", p=P
                            ),
                            obt[og][:, ds(oi - 1, 2), :],
                        )
                return go

            for og in range(2):
                for oi in range(4):
                    units.append(ou(og, oi))
            return units

        phase_a(0, 0)
        phase_a(0, 1)

        push_at = {
            (0, 0): [("a", 1)],
            (1, 0): [("a", 2)],
            (2, 0): [("a", 3), ("o", 0)],
            (3, 0): [("o", 1)],
            (3, 2): [("o", 2)],
        }

        pend_norm = None
        for qb in range(4):
            for m in range(4):
                for kind, idx in push_at.get((qb, m), []):
                    if kind == "o":
                        while any(t[0] == "n" and t[1] == idx for t, _ in filler):
                            filler.popleft()[1]()
                    if kind == "a":
                        for jp in (0, 1):
                            for u in phase_a_units(idx, jp):
                                filler.append((("a", idx), u))
                    else:
                        for u in o_units(idx):
                            filler.append((("o", idx), u))
                if m == 0:
                    flush_a(qb)
                pvt = kt_loop(qb, m)
                rc, araw = norm_a(qb, m, pvt)
                if pend_norm is not None:
                    nq, nm, nrc, nar = pend_norm
                    for u in reversed(norm_b_units(nq, nm, nrc, nar)):
                        filler.appendleft((("n", nq), u))
                pend_norm = (qb, m, rc, araw)
        while filler:
            filler.popleft()[1]()
        norm_b(*pend_norm)
        for u in o_units(3):
            u()

        if CFG.get("debug"):
            dQT, dKT, dVP, dHOPT = CFG["_dbg"]
            nc.sync.dma_start(dQT.ap(), QT if need_f16qt else QT8)
            nc.sync.dma_start(dKT.ap(), KT if need_f16qt else KT8)
            nc.sync.dma_start(dHOPT.ap(), HOPT)
            nc.sync.dma_start(dVP.ap(), VP)


# ---------------- host side ----------------

def _qk_perm():
    perm = []
    for j in range(4):
        for h4 in range(4):
            h = (j // 2) * 4 + h4
            par = j % 2
            perm.extend(h * 64 + 2 * np.arange(32) + par)
    return np.array(perm)


_PERM = _qk_perm()


def _host_tables():
    import ml_dtypes

    F16n = np.float16
    i = np.arange(32, dtype=np.float32)
    inv_freq = (THETA ** (2.0 * i / DK)).astype(np.float32)
    t = np.arange(S, dtype=np.float32)
    ang = t[None, :] / inv_freq[:, None]          # [32, S]
    sc = 1.0 / WSC if CFG["proj_hilo"] else 1.0
    cosP = np.tile(sc * np.cos(ang), (4, 1)).astype(F16n)   # [128, S]
    sinP = np.tile(sc * np.sin(ang), (4, 1)).astype(F16n)
    ident = np.eye(P, dtype=F16n)
    kk = np.arange(P)[:, None]
    qq = np.arange(512)[None, :]
    umask = (kk > qq).astype(F16n)                # [128, 512]
    negI = (NEG * np.eye(P)).astype(F16n)
    F8n = ml_dtypes.float8_e4m3fn
    negI8 = np.zeros((P, 2, P), dtype=np.float32)
    umask8 = np.zeros((P, 2, 512), dtype=np.float32)
    for ii in range(2):
        for p in range(64):
            negI8[p, ii, ii * 64 + p] = NEG8
            umask8[p, ii, :] = ((ii * 64 + p) > qq[0]).astype(np.float32)
    return cosP, sinP, ident, umask, negI, umask8.astype(F8n), negI8.astype(F8n)


def make_in_maps(x, Wq, Wk, Wv, Wo):
    import ml_dtypes

    F16n = np.float16
    F8n = ml_dtypes.float8_e4m3fn
    cosP, sinP, ident, umask, negI, umask8, negI8 = _host_tables()
    in_maps = []
    for c in range(8):
        b, hh = c // 2, c % 2
        sl = slice(hh * HD, (hh + 1) * HD)
        xT = np.ascontiguousarray(x[b].T)                      # [1024, 2048]
        wq = Wq[sl, :][_PERM]
        wk = Wk[sl, :][_PERM]
        wv = Wv[sl, :]
        wo = Wo[:, sl]                                         # [1024, 512]
        woh = np.ascontiguousarray(wo.T).reshape(4, P, D)
        m = {
            "xh": xT.reshape(8, P, S).astype(F16n),
            "wqh": np.ascontiguousarray(wq.T).reshape(8, P, HD).astype(F16n),
            "wkh": np.ascontiguousarray(wk.T).reshape(8, P, HD).astype(F16n),
            "wvh": np.ascontiguousarray(wv.T).reshape(8, P, HD).astype(F16n),
            "woh": woh.astype(F16n),
            "cosP": cosP, "sinP": sinP, "ident": ident,
            "umask": umask, "negI": negI, "umask8": umask8, "negI8": negI8,
        }
        if CFG["proj_hilo"]:
            def drfmt(a, n):
                return np.ascontiguousarray(a).reshape(4, 2, P, n).transpose(2, 0, 1, 3)

            def hilo8(a, n, scale):
                a = drfmt(a, n) * scale
                hi = a.astype(F8n)
                lo = (a - hi.astype(np.float32)).astype(F8n)
                return hi, lo

            m["x8h"], m["x8l"] = hilo8(xT, S, 1.0)
            m["wq8h"], m["wq8l"] = hilo8(wq.T, HD, WSC)
            m["wk8h"], m["wk8l"] = hilo8(wk.T, HD, WSC)
            m["wv8h"], m["wv8l"] = hilo8(wv.T, HD, WSC)
        in_maps.append(m)
    return in_maps


def gather_out(core_outs):
    out = np.empty((B, S, D), dtype=np.float32)
    for b in range(B):
        out[b] = (core_outs[2 * b]["outT"] + core_outs[2 * b + 1]["outT"]).T
    return out


_NC_CACHE = {}


def kernel(x, Wq, Wk, Wv, Wo):
    x = np.asarray(x, dtype=np.float32)
    Wq = np.asarray(Wq, dtype=np.float32)
    Wk = np.asarray(Wk, dtype=np.float32)
    Wv = np.asarray(Wv, dtype=np.float32)
    Wo = np.asarray(Wo, dtype=np.float32)
    if "nc" not in _NC_CACHE:
        _NC_CACHE["nc"] = build_attention_nc()
    nc = _NC_CACHE["nc"]
    in_maps = make_in_maps(x, Wq, Wk, Wv, Wo)
    res = run_bass_kernel_spmd(nc, in_maps, core_ids=list(range(8)))
    return gather_out(res.results)
